# revision 49
# baseline (speedup 1.0000x reference)
"""MultiHeadAttention (RoPE + QK-RMSNorm, non-causal) on 8 trn2 NeuronCores.

Sharding: batch (2) x head-groups (4 heads each) -> 8 cores. Each core:
  - QKV projection for its 4 heads (768 output channels) from x[b] (full seq)
  - RoPE + QK-RMSNorm fused via channel permutation + per-row scale matmuls
  - full 2048x2048 attention for 4 heads (flash-style, scores transposed,
    softmax without max-subtraction: RMS-normed logits are bounded by 8)
  - output projection partial (its 256 channels of the 1024-ch contraction)
Host: sums the 4 partials per batch (fp16 device outputs) and adds b_out.

Performance structure (tuned against perfetto + HAM traces):
  - every matmul operand is bf16 (fp16 streams at HALF the PE rate on TRN2;
    bf16/f32r run the full 2.4 GHz column rate)
  - the attention j-loop is ScalarE-bound (EXP at (N+352)/1.2 ns); emission
    is software-pipelined (scores[j+1] before PV[j]) and everything movable
    (pair1 QKV chains, its RMS stats / gamma broadcasts / V transposes /
    rope, and the split output projection) drips into the windows as PE
    filler so the PE HAM clock stays at 8/8
  - Ln and Exp live in different ACT table sets (~1.3us reload per switch),
    so the RMS-norm batches all Lns, then all Exps
  - the output projection is split per pair: pair0's half accumulates to
    bf16 SBUF partials during its own attention, pair1 adds them back
"""
import math
import ml_dtypes
import numpy as np

import concourse.bass as bass
from concourse import bacc
import concourse.mybir as mybir
import concourse.tile as tile
from concourse.bass_utils import run_bass_kernel_spmd
from concourse.masks import make_identity

F32 = mybir.dt.float32
F32R = mybir.dt.float32r
F16 = mybir.dt.float16
BF16 = mybir.dt.bfloat16
AF = mybir.ActivationFunctionType

B, L, C, H, D = 2, 2048, 1024, 16, 64
NCORES = 8
ROPE_THETA = 10000.0
RMS_EPS = 1e-6
NPAIR = 2        # head pairs per core
LQB = 512        # q block size
NQB = L // LQB   # 4 q blocks
NKV = L // 128   # 16 kv chunks


def _build_program():
    nc = bacc.Bacc("TRN2", target_bir_lowering=False, debug=False)

    xt_d = nc.dram_tensor("xt", [C, L], BF16, kind="ExternalInput")
    wq_d = nc.dram_tensor("wq", [C, 768], BF16, kind="ExternalInput")
    bias_d = nc.dram_tensor("bias6", [6, 128], F32, kind="ExternalInput")
    cos_d = nc.dram_tensor("cost", [128, L], BF16, kind="ExternalInput")
    sin_d = nc.dram_tensor("sint", [128, L], BF16, kind="ExternalInput")
    ind_d = nc.dram_tensor("ind", [128, 4], BF16, kind="ExternalInput")
    gind_d = nc.dram_tensor("gind", [4, 4 * 128], F32R, kind="ExternalInput")
    ones_d = nc.dram_tensor("ones128", [128, 64], F32R, kind="ExternalInput")
    wo_d = nc.dram_tensor("wo", [256, 1024], BF16, kind="ExternalInput")
    out_d = nc.dram_tensor("out", [L, C], F16, kind="ExternalOutput")

    with tile.TileContext(nc) as tc:
        with tc.tile_pool(name="const", bufs=1) as cp:
            # ---- constant tiles (DMAs for late-use consts issued after the
            # hot-path wq/xt loads so the first QKV chains start ASAP) ----
            cos_t = cp.tile([128, L], BF16, tag="cos")
            sin_t = cp.tile([128, L], BF16, tag="sin")
            ind_t = cp.tile([128, 4], BF16, tag="ind")
            gind_t = cp.tile([4, 4 * 128], F32R, tag="gind")
            ones_t = cp.tile([128, 64], F32R, tag="ones")
            wo_t = [cp.tile([128, 1024], BF16, tag=f"wo{p}", name=f"wo{p}")
                    for p in range(2)]
            bias_t = cp.tile([128, 6], F32, tag="bias")
            lnb_t = cp.tile([4, 1], F32, tag="lnb")
            nc.vector.memset(lnb_t[:, :], 0.0)
            nc.vector.memset(lnb_t[0:2, :], -math.log(8.0))
            eps_t = cp.tile([4, 1], F32, tag="eps")
            nc.vector.memset(eps_t[:, :], RMS_EPS)
            ident = cp.tile([128, 128], BF16, tag="ident")
            make_identity(nc, ident[:, :])
            onecol = cp.tile([128, 2], F32, tag="onecol")
            nc.vector.memset(onecol[:, :], 1.0)

            # xw below the chunk pools on the right stack: released last
            # (from inside the attention filler, after chkD)
            xw = tc.alloc_tile_pool(name="xw", bufs=1, side="right")
            # E/O/V chunks per pair: rows of E = [qA_e, qB_e, kA_e, kB_e]
            chkD = tc.alloc_tile_pool(name="chkD", bufs=1, side="right")
            chk = tc.alloc_tile_pool(name="chk", bufs=1, side="right")
            chunks = [
                (chkD if i in (3, 4, 5) else chk).tile(
                    [128, L], BF16,
                    tag=f"c{i}", name=f"c{i}") for i in range(6)]

            # ---- phase 1: fused QKV projection (pair0 chunks up front;
            # pair1's chains run as attention filler, xw stays alive) ----
            with tc.tile_pool(name="psq", bufs=2, space="PSUM") as psq:
                wq_sb = []
                for cc in range(8):
                    wqi = xw.tile([128, 768], BF16, tag=f"w{cc}", name=f"w{cc}")
                    nc.sync.dma_start(out=wqi, in_=wq_d[cc * 128:(cc + 1) * 128, :])
                    wq_sb.append(wqi)
                nc.sync.dma_start(out=bias_t, in_=bias_d[:, :].transpose([1, 0]))
                xt_sb = [[None] * 4 for _ in range(8)]
                for lq in range(4):
                    for cc in range(8):
                        xti = xw.tile([128, 512], BF16, tag=f"x{cc}_{lq}",
                                      name=f"x{cc}_{lq}")
                        nc.sync.dma_start(
                            out=xti,
                            in_=xt_d[cc * 128:(cc + 1) * 128,
                                     lq * 512:(lq + 1) * 512])
                        xt_sb[cc][lq] = xti
                # late-use constants after the hot path
                nc.sync.dma_start(out=ind_t, in_=ind_d[:, :])
                nc.sync.dma_start(out=cos_t, in_=cos_d[:, :])
                nc.sync.dma_start(out=sin_t, in_=sin_d[:, :])
                nc.sync.dma_start(out=gind_t, in_=gind_d[:, :])
                nc.sync.dma_start(out=ones_t, in_=ones_d[:, :])
                for p in range(2):
                    nc.sync.dma_start(out=wo_t[p],
                                      in_=wo_d[p * 128:(p + 1) * 128, :])
                for lq in range(4):
                    for oc in range(3):
                        ps = psq.tile([128, 512], F32, tag="ps")
                        for cc in range(8):
                            nc.tensor.matmul(
                                ps[:, :],
                                wq_sb[cc][:, oc * 128:(oc + 1) * 128],
                                xt_sb[cc][lq][:, :],
                                start=(cc == 0), stop=(cc == 7),
                            )
                        nc.vector.tensor_scalar_add(
                            chunks[oc][:, lq * 512:(lq + 1) * 512],
                            ps[:, :], bias_t[:, oc:oc + 1])

            # long-lived attention operands (allocated after xw released)
            lv = tc.alloc_tile_pool(name="live", bufs=1)
            qT, kT, vseq = [], [], []
            for p in range(NPAIR):
                qT.append(lv.tile([128, L], BF16, tag=f"qT{p}", name=f"qT{p}"))
                kT.append(lv.tile([128, L], BF16, tag=f"kT{p}", name=f"kT{p}"))
                vseq.append([lv.tile([128, 130], BF16, tag=f"vs{p}_{lw}",
                                     name=f"vs{p}_{lw}") for lw in range(NKV)])
            # normalized attention outputs, per (pair, q-block)
            oq = [[lv.tile([128, LQB], BF16, tag=f"oq{p}_{qb}",
                           name=f"oq{p}_{qb}") for qb in range(NQB)]
                  for p in range(NPAIR)]

            # ---- phase 2: pair0 rope + rmsnorm + relocation + v transpose
            # (pair1's prep is deferred into pair0's attention as filler) ----
            dfr = tc.alloc_tile_pool(name="dfr", bufs=1)
            M_sb = [dfr.tile([128, L], BF16, tag=f"Msb{k}", name=f"Msb{k}")
                    for k in range(2)]
            with (
                tc.tile_pool(name="tmp", bufs=1) as tp,
                tc.tile_pool(name="psp", bufs=2, space="PSUM") as psp,
            ):
                E, O, V = chunks[0], chunks[1], chunks[2]
                sqE = tp.tile([128, L], BF16, tag="bigA")
                nc.vector.tensor_mul(sqE[:, :], E[:, :], E[:, :])
                sqO = tp.tile([128, L], BF16, tag="bigB")
                nc.vector.tensor_mul(sqO[:, :], O[:, :], O[:, :])
                # batch all Ln calls, then all Exp calls: Ln and Exp live in
                # different ACT table sets, and every alternation costs a
                # ~1.3us ACT_TABLE_LOAD
                lnvs = []
                for lw in range(4):
                    ps4 = psp.tile([4, 512], F32, tag="ps4")
                    nc.tensor.matmul(ps4[:, :], ind_t[:, :],
                                     sqE[:, lw * 512:(lw + 1) * 512],
                                     start=True, stop=False)
                    nc.tensor.matmul(ps4[:, :], ind_t[:, :],
                                     sqO[:, lw * 512:(lw + 1) * 512],
                                     start=False, stop=True)
                    lnv = tp.tile([4, 512], F32, tag=f"lnv{lw}",
                                  name=f"lnv{lw}")
                    nc.scalar.activation(lnv[:, :], ps4[:, :], AF.Ln,
                                         scale=1.0 / 64.0, bias=eps_t[:, 0:1])
                    lnvs.append(lnv)
                invrs = []
                for lw in range(4):
                    iv = tp.tile([4, 512], F32R, tag=f"inv{lw}",
                                 name=f"inv{lw}")
                    nc.scalar.activation(iv[:, :], lnvs[lw][:, :], AF.Exp,
                                         scale=-0.5, bias=lnb_t[:, 0:1])
                    invrs.append(iv)
                # rope now (bf16, 3 temps: A, B, C)
                t1c = tp.tile([128, L], BF16, tag="bigC")
                nc.vector.tensor_mul(t1c[:, :], E[:, :], cos_t[:, :])
                t2s = tp.tile([128, L], BF16, tag="bigB")
                nc.vector.tensor_mul(t2s[:, :], O[:, :], sin_t[:, :])
                rE = tp.tile([128, L], BF16, tag="bigA")
                nc.vector.tensor_sub(rE[:, :], t1c[:, :], t2s[:, :])
                t1s = tp.tile([128, L], BF16, tag="bigC")
                nc.vector.tensor_mul(t1s[:, :], E[:, :], sin_t[:, :])
                t2c = tp.tile([128, L], BF16, tag="bigB")
                nc.vector.tensor_mul(t2c[:, :], O[:, :], cos_t[:, :])
                rO = t1s
                nc.vector.tensor_add(rO[:, :], t1s[:, :], t2c[:, :])
                sE = tp.tile([128, L], BF16, tag="sc16E")
                sO = tp.tile([128, L], BF16, tag="sc16O")
                for kind, (rt, st) in enumerate(((rE, sE), (rO, sO))):
                    gsl = gind_t[:, kind * 128:(kind + 1) * 128]
                    for lw in range(4):
                        mm = psp.tile([128, 512], F32, tag="mps")
                        nc.tensor.matmul(mm[:, :], gsl,
                                         invrs[lw][:, :],
                                         start=True, stop=True)
                        nc.vector.tensor_mul(
                            st[:, lw * 512:(lw + 1) * 512],
                            rt[:, lw * 512:(lw + 1) * 512], mm[:, :])
                for blk in range(2):
                    nc.sync.dma_start(out=qT[0][blk * 64:blk * 64 + 32, :],
                                      in_=sE[blk * 32:(blk + 1) * 32, :])
                    nc.sync.dma_start(out=qT[0][blk * 64 + 32:blk * 64 + 64, :],
                                      in_=sO[blk * 32:(blk + 1) * 32, :])
                    nc.sync.dma_start(out=kT[0][blk * 64:blk * 64 + 32, :],
                                      in_=sE[64 + blk * 32:64 + (blk + 1) * 32, :])
                    nc.sync.dma_start(out=kT[0][blk * 64 + 32:blk * 64 + 64, :],
                                      in_=sO[64 + blk * 32:64 + (blk + 1) * 32, :])
                # v transpose -> vseq [l,130]: [vA(64) 1 vB(64) 1]
                for lw in range(NKV):
                    pt = psp.tile([128, 128], BF16, tag="ptr")
                    nc.tensor.transpose(pt[:, :],
                                        V[:, lw * 128:(lw + 1) * 128],
                                        ident[:, :])
                    vv = vseq[0][lw].rearrange("a (h x) -> a h x", h=2)
                    nc.vector.tensor_copy(
                        vv[:, :, 0:64],
                        pt[:, :].rearrange("a (h x) -> a h x", h=2))
                    nc.vector.tensor_copy(vv[:, :, 64], onecol[:, :])

            chk.release()
            # pair0 half of the output projection, bf16 partials
            pav = tc.alloc_tile_pool(name="pav", bufs=1)
            pA = [[pav.tile([128, 512], BF16, tag=f"pA{qb}_{i}",
                            name=f"pA{qb}_{i}") for i in range(8)]
                  for qb in range(NQB)]
            dfr2 = tc.alloc_tile_pool(name="dfr2", bufs=1)
            rA = dfr2.tile([128, L], BF16, tag="rA")
            rB = dfr2.tile([128, L], BF16, tag="rB")
            sE1 = dfr2.tile([128, L], BF16, tag="sE1")
            sO1 = dfr2.tile([128, L], BF16, tag="sO1")
            lnv1 = [dfr2.tile([4, 512], F32, tag=f"lnv1_{lw}",
                              name=f"lnv1_{lw}") for lw in range(4)]
            iv1 = [dfr2.tile([4, 512], F32R, tag=f"iv1_{lw}",
                             name=f"iv1_{lw}") for lw in range(4)]
            E1, O1, V1 = chunks[3], chunks[4], chunks[5]

            # pair1 prep, dripped one item per attention window of pair0.
            # PSUM scratch comes from the attention-phase "po" slots, so the
            # closures must run inside the attention pool scope.
            # Filler ordering matters: engines execute in program order, so a
            # filler op whose producer (on another engine) hasn't run yet
            # head-blocks every later op on its engine — including the EXP
            # stream. Squares go first; ACT-free transpose items pad the gap
            # until the squares have certainly retired; only then the
            # Ln/Exp stats; then the gamma broadcasts; then rope.
            def _mk_deferred(pso_tile):
                u = []

                def qkv1(oc, lq):
                    ps = pso_tile([128, 512], F32)
                    for cc in range(8):
                        nc.tensor.matmul(
                            ps[:, :],
                            wq_sb[cc][:, oc * 128:(oc + 1) * 128],
                            xt_sb[cc][lq][:, :],
                            start=(cc == 0), stop=(cc == 7),
                        )
                    nc.vector.tensor_scalar_add(
                        chunks[oc][:, lq * 512:(lq + 1) * 512],
                        ps[:, :], bias_t[:, oc:oc + 1])
                for oc in (3, 4, 5):
                    for lq in range(4):
                        u.append((1700, lambda oc=oc, lq=lq: qkv1(oc, lq)))

                u.append((0, lambda: nc.vector.tensor_mul(rA[:, :], E1[:, :], E1[:, :])))
                u.append((0, lambda: nc.vector.tensor_mul(rB[:, :], O1[:, :], O1[:, :])))

                def stats_ln():
                    # one item: all four Ln calls back-to-back (1 table load)
                    for lw in range(4):
                        ps4 = pso_tile([4, 512], F32)
                        nc.tensor.matmul(ps4[:, :], ind_t[:, :],
                                         rA[:, lw * 512:(lw + 1) * 512],
                                         start=True, stop=False)
                        nc.tensor.matmul(ps4[:, :], ind_t[:, :],
                                         rB[:, lw * 512:(lw + 1) * 512],
                                         start=False, stop=True)
                        nc.scalar.activation(lnv1[lw][:, :], ps4[:, :], AF.Ln,
                                             scale=1.0 / 64.0,
                                             bias=eps_t[:, 0:1])

                def stats_exp():
                    for lw in range(4):
                        nc.scalar.activation(iv1[lw][:, :], lnv1[lw][:, :],
                                             AF.Exp, scale=-0.5,
                                             bias=lnb_t[:, 0:1])

                def msb(kind, lw):
                    gsl = gind_t[:, (2 + kind) * 128:(2 + kind + 1) * 128]
                    mm = pso_tile([128, 512], F32)
                    nc.tensor.matmul(mm[:, :], gsl, iv1[lw][:, :],
                                     start=True, stop=True)
                    nc.vector.tensor_copy(
                        M_sb[kind][:, lw * 512:(lw + 1) * 512], mm[:, :])

                def vtr(lw):
                    pt = pso_tile([128, 128], BF16)
                    nc.tensor.transpose(pt[:, :], V1[:, lw * 128:(lw + 1) * 128],
                                        ident[:, :])
                    vv = vseq[1][lw].rearrange("a (h x) -> a h x", h=2)
                    nc.vector.tensor_copy(
                        vv[:, :, 0:64],
                        pt[:, :].rearrange("a (h x) -> a h x", h=2))
                    nc.vector.tensor_copy(vv[:, :, 64], onecol[:, :])

                for lw in range(10):
                    u.append((100, lambda lw=lw: vtr(lw)))
                u.append((900, stats_ln))
                for lw in range(10, 13):
                    u.append((100, lambda lw=lw: vtr(lw)))
                u.append((0, stats_exp))
                for lw in range(13, NKV):
                    u.append((100, lambda lw=lw: vtr(lw)))
                for kind in range(2):
                    for lw in range(4):
                        u.append((250, lambda k=kind, lw=lw: msb(k, lw)))

                u.append((0, lambda: nc.vector.tensor_mul(rA[:, :], E1[:, :], cos_t[:, :])))
                u.append((0, lambda: nc.vector.tensor_mul(rB[:, :], O1[:, :], sin_t[:, :])))
                u.append((0, lambda: nc.vector.tensor_sub(rA[:, :], rA[:, :], rB[:, :])))
                u.append((0, lambda: nc.vector.tensor_mul(sE1[:, :], rA[:, :], M_sb[0][:, :])))
                u.append((0, lambda: nc.vector.tensor_mul(rA[:, :], E1[:, :], sin_t[:, :])))
                u.append((0, lambda: nc.vector.tensor_mul(rB[:, :], O1[:, :], cos_t[:, :])))
                u.append((0, lambda: nc.vector.tensor_add(rA[:, :], rA[:, :], rB[:, :])))
                u.append((0, lambda: nc.vector.tensor_mul(sO1[:, :], rA[:, :], M_sb[1][:, :])))
                for blk in range(2):
                    for args in (
                        (qT[1][blk * 64:blk * 64 + 32, :], sE1[blk * 32:(blk + 1) * 32, :]),
                        (qT[1][blk * 64 + 32:blk * 64 + 64, :], sO1[blk * 32:(blk + 1) * 32, :]),
                        (kT[1][blk * 64:blk * 64 + 32, :], sE1[64 + blk * 32:64 + (blk + 1) * 32, :]),
                        (kT[1][blk * 64 + 32:blk * 64 + 64, :], sO1[64 + blk * 32:64 + (blk + 1) * 32, :]),
                    ):
                        u.append((0, lambda a=args: nc.sync.dma_start(out=a[0], in_=a[1])))
                u.append((0, lambda: chkD.release()))
                u.append((0, lambda: xw.release()))
                return u

            # ---- phase 3: attention + fused output projection ----
            # Per (pair, q-block of 512): flash loop over 16 kv chunks.
            # Emission is software-pipelined (scores[j+1] ahead of PV[j]) so
            # the PE stream never head-blocks on the EXP; the out-projection
            # for q-block qb is dripped into the following windows as PE
            # filler (keeps the HAM clock warm).
            with (
                tc.tile_pool(name="exp", bufs=4) as xp,
                tc.tile_pool(name="den", bufs=2) as dn,
                tc.tile_pool(name="oev", bufs=2) as oe,
                tc.tile_pool(name="psa", bufs=2, space="PSUM") as psa,
                tc.tile_pool(name="psv", bufs=1, space="PSUM") as psv,
                tc.tile_pool(name="pso", bufs=2, space="PSUM") as pso,
            ):
                _poctr = [0]

                def _po_tile(shape, dt):
                    _poctr[0] += 1
                    return pso.tile(shape, dt, tag="po",
                                    name=f"pof{_poctr[0]}")

                filler = _mk_deferred(_po_tile)

                # out-projection split across pairs: pair0's half runs as
                # filler during its own attention (PSUM -> bf16 SBUF
                # partials), pair1's half adds them back. Both phases get a
                # real matmul per window, which keeps the PE HAM clock warm.
                def mk_proj0(qb):
                    def one(i, lw, n):
                        def run():
                            po = _po_tile([128, 512], F32)
                            nc.tensor.matmul(
                                po[:, :],
                                oq[0][qb][:, lw * 128:(lw + 1) * 128],
                                wo_t[0][:, n * 512:(n + 1) * 512],
                                start=True, stop=True)
                            nc.vector.tensor_copy(pA[qb][i][:, :], po[:, :])
                        return run
                    return [(450, one(i, lw, n))
                            for i, (lw, n) in enumerate(
                                (lw, n) for lw in range(4) for n in range(2))]

                def mk_proj1(qb):
                    def one(i, lw, n):
                        def run():
                            po = _po_tile([128, 512], F32)
                            nc.tensor.matmul(
                                po[:, :],
                                oq[1][qb][:, lw * 128:(lw + 1) * 128],
                                wo_t[1][:, n * 512:(n + 1) * 512],
                                start=True, stop=True)
                            ov = oe.tile([128, 512], F16, tag="ov")
                            nc.vector.tensor_add(ov[:, :], po[:, :],
                                                 pA[qb][i][:, :])
                            nc.sync.dma_start(
                                out=out_d[qb * LQB + lw * 128:
                                          qb * LQB + (lw + 1) * 128,
                                          n * 512:(n + 1) * 512],
                                in_=ov[:, :])
                        return run
                    return [(450, one(i, lw, n))
                            for i, (lw, n) in enumerate(
                                (lw, n) for lw in range(4) for n in range(2))]

                _dctr = [0]
                # bridge the phase2 -> attention transition: give the PE two
                # ready QKV chains to chew while the rope relocation DMAs
                # land, so the HAM clock never sees an idle window
                for _ in range(2):
                    filler.pop(0)[1]()

                for p in range(NPAIR):
                    if p == 1:
                        while filler:
                            filler.pop(0)[1]()
                    for qb in range(NQB):
                        q0 = qb * LQB
                        oA = psv.tile([65, LQB], F32, tag="oA")
                        oB = psv.tile([65, LQB], F32, tag="oB")

                        def emit_scores(j):
                            sAB = psa.tile([128, 2 * LQB], F32, tag="sAB")
                            nc.tensor.matmul(
                                sAB[:, 0:LQB],
                                kT[p][0:64, j * 128:(j + 1) * 128],
                                qT[p][0:64, q0:q0 + LQB],
                                start=True, stop=True, tile_position=(0, 0))
                            nc.tensor.matmul(
                                sAB[:, LQB:2 * LQB],
                                kT[p][64:128, j * 128:(j + 1) * 128],
                                qT[p][64:128, q0:q0 + LQB],
                                start=True, stop=True, tile_position=(64, 0))
                            return sAB

                        def emit_pv(j, eAB):
                            nc.tensor.matmul(
                                oA[:, :], vseq[p][j][:, 0:65],
                                eAB[:, 0:LQB],
                                start=(j == 0), stop=(j == NKV - 1))
                            nc.tensor.matmul(
                                oB[:, :], vseq[p][j][:, 65:130],
                                eAB[:, LQB:2 * LQB],
                                start=(j == 0), stop=(j == NKV - 1))

                        prev_e = None
                        for j in range(NKV):
                            sAB = emit_scores(j)
                            if prev_e is not None:
                                emit_pv(j - 1, prev_e)
                            eAB = xp.tile([128, 2 * LQB], BF16, tag="eAB")
                            nc.scalar.activation(eAB[:, :], sAB[:, :], AF.Exp)
                            prev_e = eAB
                            got = 0
                            if filler:
                                w, fn = filler.pop(0)
                                fn()
                                got += w
                            if len(filler) > 28:
                                w, fn = filler.pop(0)
                                fn()
                                got += w
                        emit_pv(NKV - 1, prev_e)

                        # normalize: o / denom (denom = row 64); the
                        # ones-row matmul broadcast beats a DMA+GpSimd route
                        # (the DMA round-trip stalls each q-block boundary)
                        for hd, oo in enumerate((oA, oB)):
                            den = dn.tile([65, LQB], F32R, tag="den")
                            nc.vector.tensor_copy(den[64:65, :], oo[64:65, :])
                            dbc = pso.tile([64, LQB], F32, tag="po")
                            nc.tensor.matmul(
                                dbc[:, :], ones_t[64:65, :],
                                den[64:65, :], start=True, stop=True)
                            rcb = dn.tile([64, LQB], F32, tag="rcb")
                            nc.vector.reciprocal_approx_fast(rcb[:, :], dbc[:, :])
                            onrm = dn.tile([64, LQB], BF16, tag="onrm")
                            nc.vector.tensor_mul(onrm[:, :], oo[0:64, :], rcb[:, :])
                            nc.sync.dma_start(
                                out=oq[p][qb][hd * 64:(hd + 1) * 64, :],
                                in_=onrm[:, :])
                        filler.extend(
                            (mk_proj0 if p == 0 else mk_proj1)(qb))
                while filler:
                    filler.pop(0)[1]()


            dfr2.release()
            pav.release()
            dfr.release()
            lv.release()
    nc.compile()
    return nc


_PROG = None


def _get_program():
    global _PROG
    if _PROG is None:
        _PROG = _build_program()
    return _PROG


def _core_inputs(core, x, w_qkv, b_qkv, w_out, q_gamma, k_gamma,
                 cos_tab, sin_tab, ind, ones128):
    b = core // 4
    hb = (core % 4) * 4
    # row permutation of w_qkv for this core
    perm = []
    for p in range(NPAIR):
        hA, hB = hb + 2 * p, hb + 2 * p + 1
        for h in (hA, hB):                      # E chunk: q evens, k evens
            perm += [h * 64 + 2 * j for j in range(32)]
        for h in (hA, hB):
            perm += [1024 + h * 64 + 2 * j for j in range(32)]
        for h in (hA, hB):                      # O chunk
            perm += [h * 64 + 2 * j + 1 for j in range(32)]
        for h in (hA, hB):
            perm += [1024 + h * 64 + 2 * j + 1 for j in range(32)]
        for h in (hA, hB):                      # V chunk
            perm += [2048 + h * 64 + c for c in range(64)]
    perm = np.asarray(perm)
    w_local = w_qkv[perm]                       # [768, 1024]
    wq = np.ascontiguousarray(w_local.T)        # [1024, 768]
    bias6 = np.ascontiguousarray(b_qkv[perm].reshape(6, 128))

    # gamma-scaled indicator lhsT [4, 4*128]: (p, kind) -> [4, 128]
    gind = np.zeros((4, 4 * 128), np.float32)
    for p in range(NPAIR):
        for kind in range(2):                   # 0=E(evens), 1=O(odds)
            blk = (2 * p + kind) * 128
            for r in range(128):
                g = r // 32
                h = hb + 2 * p + (g % 2)
                ch = 2 * (r % 32) + kind
                gam = q_gamma[h, ch] if g < 2 else k_gamma[h, ch]
                gind[g, blk + r] = gam

    # w_out slice: [256, 1024]
    wo = np.empty((256, 1024), np.float32)
    for p in range(NPAIR):
        for i, h in enumerate((hb + 2 * p, hb + 2 * p + 1)):
            wo[p * 128 + i * 64:p * 128 + (i + 1) * 64, :] = \
                w_out[:, h * 64:(h + 1) * 64].T

    return {
        "xt": np.ascontiguousarray(x[b].T).astype(ml_dtypes.bfloat16),
        "wq": wq.astype(ml_dtypes.bfloat16),
        "bias6": bias6,
        "cost": cos_tab,
        "sint": sin_tab,
        "ind": ind.astype(ml_dtypes.bfloat16),
        "gind": gind,
        "ones128": ones128,
        "wo": wo.astype(ml_dtypes.bfloat16),
    }


def kernel(x, w_qkv, b_qkv, w_out, b_out, q_gamma, k_gamma, _trace=False):
    x = np.asarray(x, np.float32)
    w_qkv = np.asarray(w_qkv, np.float32)
    b_qkv = np.asarray(b_qkv, np.float32)
    w_out = np.asarray(w_out, np.float32)
    b_out = np.asarray(b_out, np.float32)
    q_gamma = np.asarray(q_gamma, np.float32)
    k_gamma = np.asarray(k_gamma, np.float32)

    inv_freq = (1.0 / ROPE_THETA ** (np.arange(32, dtype=np.float64) / 32.0))
    ang = np.arange(L, dtype=np.float64)[None, :] * \
        np.tile(inv_freq, 4)[:, None]          # [128, L], row r -> freq r%32
    cos_tab = np.cos(ang).astype(ml_dtypes.bfloat16)
    sin_tab = np.sin(ang).astype(ml_dtypes.bfloat16)
    ind = np.zeros((128, 4), np.float32)
    for r in range(128):
        ind[r, r // 32] = 1.0
    ones128 = np.ones((128, 64), np.float32)

    nc = _get_program()
    in_maps = [_core_inputs(c, x, w_qkv, b_qkv, w_out, q_gamma, k_gamma,
                            cos_tab, sin_tab, ind, ones128)
               for c in range(NCORES)]
    r = run_bass_kernel_spmd(nc, in_maps, list(range(NCORES)), trace=_trace)
    out = np.zeros((B, L, C), np.float32)
    for c in range(NCORES):
        out[c // 4] += r.results[c]["out"].astype(np.float32)
    out += b_out[None, None, :]
    if _trace:
        kernel._last_results = r
    return out



# revision 50
# speedup vs baseline: 1.0838x; 1.0838x over previous
"""MultiHeadAttention (RoPE + QK-RMSNorm, non-causal) on 8 trn2 NeuronCores.

Sharding: batch (2) x head-groups (4 heads each) -> 8 cores. Each core:
  - QKV projection for its 4 heads (768 output channels) from x[b] (full seq)
  - RoPE + QK-RMSNorm fused via channel permutation + per-row scale matmuls
  - full 2048x2048 attention for 4 heads (flash-style, scores transposed,
    softmax without max-subtraction: RMS-normed logits are bounded by 8)
  - output projection partial (its 256 channels of the 1024-ch contraction)
Host: sums the 4 partials per batch (fp16 device outputs) and adds b_out.

Performance structure (tuned against perfetto + HAM traces):
  - every matmul operand is bf16 (fp16 streams at HALF the PE rate on TRN2;
    bf16/f32r run the full 2.4 GHz column rate)
  - the attention j-loop is ScalarE-bound (EXP at (N+352)/1.2 ns); emission
    is software-pipelined (scores[j+1] before PV[j]) and everything movable
    (pair1 QKV chains, its RMS stats / gamma broadcasts / V transposes /
    rope, and the split output projection) drips into the windows as PE
    filler so the PE HAM clock stays at 8/8
  - Ln and Exp live in different ACT table sets (~1.3us reload per switch),
    so the RMS-norm batches all Lns, then all Exps
  - the output projection is split per pair: pair0's half accumulates to
    bf16 SBUF partials during its own attention, pair1 adds them back
"""
import math
import ml_dtypes
import numpy as np

import concourse.bass as bass
from concourse import bacc
import concourse.mybir as mybir
import concourse.tile as tile
from concourse.bass_utils import run_bass_kernel_spmd
from concourse.masks import make_identity

F32 = mybir.dt.float32
F32R = mybir.dt.float32r
F16 = mybir.dt.float16
BF16 = mybir.dt.bfloat16
AF = mybir.ActivationFunctionType

B, L, C, H, D = 2, 2048, 1024, 16, 64
NCORES = 8
ROPE_THETA = 10000.0
RMS_EPS = 1e-6
NPAIR = 2        # head pairs per core
LQB = 512        # q block size
NQB = L // LQB   # 4 q blocks
NKV = L // 128   # 16 kv chunks


def _build_program():
    nc = bacc.Bacc("TRN2", target_bir_lowering=False, debug=False)

    xt_d = nc.dram_tensor("xt", [C, L], BF16, kind="ExternalInput")
    wq_d = nc.dram_tensor("wq", [C, 768], BF16, kind="ExternalInput")
    bias_d = nc.dram_tensor("bias6", [6, 128], F32, kind="ExternalInput")
    cos_d = nc.dram_tensor("cost", [128, L], BF16, kind="ExternalInput")
    sin_d = nc.dram_tensor("sint", [128, L], BF16, kind="ExternalInput")
    ind_d = nc.dram_tensor("ind", [128, 4], BF16, kind="ExternalInput")
    gind_d = nc.dram_tensor("gind", [4, 4 * 128], F32R, kind="ExternalInput")
    ones_d = nc.dram_tensor("ones128", [128, 64], F32R, kind="ExternalInput")
    wo_d = nc.dram_tensor("wo", [256, 1024], BF16, kind="ExternalInput")
    out_d = nc.dram_tensor("out", [L, C], F16, kind="ExternalOutput")

    with tile.TileContext(nc) as tc:
        with tc.tile_pool(name="const", bufs=1) as cp:
            # ---- constant tiles (DMAs for late-use consts issued after the
            # hot-path wq/xt loads so the first QKV chains start ASAP) ----
            cos_t = cp.tile([128, L], BF16, tag="cos")
            sin_t = cp.tile([128, L], BF16, tag="sin")
            ind_t = cp.tile([128, 4], BF16, tag="ind")
            gind_t = cp.tile([4, 4 * 128], F32R, tag="gind")
            ones_t = cp.tile([128, 64], F32R, tag="ones")
            wo_t = [cp.tile([128, 1024], BF16, tag=f"wo{p}", name=f"wo{p}")
                    for p in range(2)]
            bias_t = cp.tile([128, 6], F32, tag="bias")
            lnb_t = cp.tile([4, 1], F32, tag="lnb")
            nc.vector.memset(lnb_t[:, :], 0.0)
            nc.vector.memset(lnb_t[0:2, :], -math.log(8.0))
            eps_t = cp.tile([4, 1], F32, tag="eps")
            nc.vector.memset(eps_t[:, :], RMS_EPS)
            ident = cp.tile([128, 128], BF16, tag="ident")
            make_identity(nc, ident[:, :])
            onecol = cp.tile([128, 2], F32, tag="onecol")
            nc.vector.memset(onecol[:, :], 1.0)

            # xw below the chunk pools on the right stack: released last
            # (from inside the attention filler, after chkD)
            xw = tc.alloc_tile_pool(name="xw", bufs=1, side="right")
            # E/O/V chunks per pair: rows of E = [qA_e, qB_e, kA_e, kB_e]
            chkD = tc.alloc_tile_pool(name="chkD", bufs=1, side="right")
            chk = tc.alloc_tile_pool(name="chk", bufs=1, side="right")
            chunks = [
                (chkD if i in (3, 4, 5) else chk).tile(
                    [128, L], BF16,
                    tag=f"c{i}", name=f"c{i}") for i in range(6)]

            # ---- phase 1: fused QKV projection (pair0 chunks up front;
            # pair1's chains run as attention filler, xw stays alive) ----
            with tc.tile_pool(name="psq", bufs=2, space="PSUM") as psq:
                wq_sb = []
                for cc in range(8):
                    wqi = xw.tile([128, 768], BF16, tag=f"w{cc}", name=f"w{cc}")
                    nc.sync.dma_start(out=wqi, in_=wq_d[cc * 128:(cc + 1) * 128, :])
                    wq_sb.append(wqi)
                nc.sync.dma_start(out=bias_t, in_=bias_d[:, :].transpose([1, 0]))
                xt_sb = [[None] * 4 for _ in range(8)]
                for lq in range(4):
                    for cc in range(8):
                        xti = xw.tile([128, 512], BF16, tag=f"x{cc}_{lq}",
                                      name=f"x{cc}_{lq}")
                        nc.sync.dma_start(
                            out=xti,
                            in_=xt_d[cc * 128:(cc + 1) * 128,
                                     lq * 512:(lq + 1) * 512])
                        xt_sb[cc][lq] = xti
                # late-use constants after the hot path
                nc.sync.dma_start(out=ind_t, in_=ind_d[:, :])
                nc.sync.dma_start(out=cos_t, in_=cos_d[:, :])
                nc.sync.dma_start(out=sin_t, in_=sin_d[:, :])
                nc.sync.dma_start(out=gind_t, in_=gind_d[:, :])
                nc.sync.dma_start(out=ones_t, in_=ones_d[:, :])
                for p in range(2):
                    nc.sync.dma_start(out=wo_t[p],
                                      in_=wo_d[p * 128:(p + 1) * 128, :])
                for lq in range(4):
                    for oc in range(3):
                        ps = psq.tile([128, 512], F32, tag="ps")
                        for cc in range(8):
                            nc.tensor.matmul(
                                ps[:, :],
                                wq_sb[cc][:, oc * 128:(oc + 1) * 128],
                                xt_sb[cc][lq][:, :],
                                start=(cc == 0), stop=(cc == 7),
                            )
                        nc.vector.tensor_scalar_add(
                            chunks[oc][:, lq * 512:(lq + 1) * 512],
                            ps[:, :], bias_t[:, oc:oc + 1])

            # long-lived attention operands (allocated after xw released)
            lv = tc.alloc_tile_pool(name="live", bufs=1)
            qT, kT, vseq = [], [], []
            for p in range(NPAIR):
                qT.append(lv.tile([128, L], BF16, tag=f"qT{p}", name=f"qT{p}"))
                kT.append(lv.tile([128, L], BF16, tag=f"kT{p}", name=f"kT{p}"))
                vseq.append([lv.tile([128, 130], BF16, tag=f"vs{p}_{lw}",
                                     name=f"vs{p}_{lw}") for lw in range(NKV)])
            # normalized attention outputs, per (pair, q-block)
            oq = [[lv.tile([128, LQB], BF16, tag=f"oq{p}_{qb}",
                           name=f"oq{p}_{qb}") for qb in range(NQB)]
                  for p in range(NPAIR)]

            # ---- phase 2: pair0 rope + rmsnorm + relocation + v transpose
            # (pair1's prep is deferred into pair0's attention as filler) ----
            dfr = tc.alloc_tile_pool(name="dfr", bufs=1)
            M_sb = [dfr.tile([128, L], BF16, tag=f"Msb{k}", name=f"Msb{k}")
                    for k in range(2)]
            with (
                tc.tile_pool(name="tmp", bufs=1) as tp,
                tc.tile_pool(name="psp", bufs=2, space="PSUM") as psp,
            ):
                E, O, V = chunks[0], chunks[1], chunks[2]
                sqE = tp.tile([128, L], BF16, tag="bigA")
                nc.vector.tensor_mul(sqE[:, :], E[:, :], E[:, :])
                sqO = tp.tile([128, L], BF16, tag="bigB")
                nc.vector.tensor_mul(sqO[:, :], O[:, :], O[:, :])
                # batch all Ln calls, then all Exp calls: Ln and Exp live in
                # different ACT table sets, and every alternation costs a
                # ~1.3us ACT_TABLE_LOAD
                lnvs = []
                for lw in range(4):
                    ps4 = psp.tile([4, 512], F32, tag="ps4")
                    nc.tensor.matmul(ps4[:, :], ind_t[:, :],
                                     sqE[:, lw * 512:(lw + 1) * 512],
                                     start=True, stop=False)
                    nc.tensor.matmul(ps4[:, :], ind_t[:, :],
                                     sqO[:, lw * 512:(lw + 1) * 512],
                                     start=False, stop=True)
                    lnv = tp.tile([4, 512], F32, tag=f"lnv{lw}",
                                  name=f"lnv{lw}")
                    nc.scalar.activation(lnv[:, :], ps4[:, :], AF.Ln,
                                         scale=1.0 / 64.0, bias=eps_t[:, 0:1])
                    lnvs.append(lnv)
                invrs = []
                for lw in range(4):
                    iv = tp.tile([4, 512], F32R, tag=f"inv{lw}",
                                 name=f"inv{lw}")
                    nc.scalar.activation(iv[:, :], lnvs[lw][:, :], AF.Exp,
                                         scale=-0.5, bias=lnb_t[:, 0:1])
                    invrs.append(iv)
                # rope now (bf16, 3 temps: A, B, C)
                t1c = tp.tile([128, L], BF16, tag="bigC")
                nc.vector.tensor_mul(t1c[:, :], E[:, :], cos_t[:, :])
                t2s = tp.tile([128, L], BF16, tag="bigB")
                nc.vector.tensor_mul(t2s[:, :], O[:, :], sin_t[:, :])
                rE = tp.tile([128, L], BF16, tag="bigA")
                nc.vector.tensor_sub(rE[:, :], t1c[:, :], t2s[:, :])
                t1s = tp.tile([128, L], BF16, tag="bigC")
                nc.vector.tensor_mul(t1s[:, :], E[:, :], sin_t[:, :])
                t2c = tp.tile([128, L], BF16, tag="bigB")
                nc.vector.tensor_mul(t2c[:, :], O[:, :], cos_t[:, :])
                rO = t1s
                nc.vector.tensor_add(rO[:, :], t1s[:, :], t2c[:, :])
                sE = tp.tile([128, L], BF16, tag="sc16E")
                sO = tp.tile([128, L], BF16, tag="sc16O")
                for kind, (rt, st) in enumerate(((rE, sE), (rO, sO))):
                    gsl = gind_t[:, kind * 128:(kind + 1) * 128]
                    for lw in range(4):
                        mm = psp.tile([128, 512], F32, tag="mps")
                        nc.tensor.matmul(mm[:, :], gsl,
                                         invrs[lw][:, :],
                                         start=True, stop=True)
                        nc.vector.tensor_mul(
                            st[:, lw * 512:(lw + 1) * 512],
                            rt[:, lw * 512:(lw + 1) * 512], mm[:, :])
                for blk in range(2):
                    nc.sync.dma_start(out=qT[0][blk * 64:blk * 64 + 32, :],
                                      in_=sE[blk * 32:(blk + 1) * 32, :])
                    nc.sync.dma_start(out=qT[0][blk * 64 + 32:blk * 64 + 64, :],
                                      in_=sO[blk * 32:(blk + 1) * 32, :])
                    nc.sync.dma_start(out=kT[0][blk * 64:blk * 64 + 32, :],
                                      in_=sE[64 + blk * 32:64 + (blk + 1) * 32, :])
                    nc.sync.dma_start(out=kT[0][blk * 64 + 32:blk * 64 + 64, :],
                                      in_=sO[64 + blk * 32:64 + (blk + 1) * 32, :])
                # v transpose -> vseq [l,130]: [vA(64) 1 vB(64) 1]
                for lw in range(NKV):
                    pt = psp.tile([128, 128], BF16, tag="ptr")
                    nc.tensor.transpose(pt[:, :],
                                        V[:, lw * 128:(lw + 1) * 128],
                                        ident[:, :])
                    vv = vseq[0][lw].rearrange("a (h x) -> a h x", h=2)
                    nc.vector.tensor_copy(
                        vv[:, :, 0:64],
                        pt[:, :].rearrange("a (h x) -> a h x", h=2))
                    nc.vector.tensor_copy(vv[:, :, 64], onecol[:, :])

            chk.release()
            # pair0 half of the output projection, bf16 partials
            pav = tc.alloc_tile_pool(name="pav", bufs=1)
            pA = [[pav.tile([128, 512], BF16, tag=f"pA{qb}_{i}",
                            name=f"pA{qb}_{i}") for i in range(8)]
                  for qb in range(NQB)]
            dfr2 = tc.alloc_tile_pool(name="dfr2", bufs=1)
            rA = dfr2.tile([128, L], BF16, tag="rA")
            rB = dfr2.tile([128, L], BF16, tag="rB")
            sE1 = dfr2.tile([128, L], BF16, tag="sE1")
            sO1 = dfr2.tile([128, L], BF16, tag="sO1")
            lnv1 = [dfr2.tile([4, 512], F32, tag=f"lnv1_{lw}",
                              name=f"lnv1_{lw}") for lw in range(4)]
            iv1 = [dfr2.tile([4, 512], F32R, tag=f"iv1_{lw}",
                             name=f"iv1_{lw}") for lw in range(4)]
            E1, O1, V1 = chunks[3], chunks[4], chunks[5]

            # pair1 prep, dripped one item per attention window of pair0.
            # PSUM scratch comes from the attention-phase "po" slots, so the
            # closures must run inside the attention pool scope.
            # Filler ordering matters: engines execute in program order, so a
            # filler op whose producer (on another engine) hasn't run yet
            # head-blocks every later op on its engine — including the EXP
            # stream. Squares go first; ACT-free transpose items pad the gap
            # until the squares have certainly retired; only then the
            # Ln/Exp stats; then the gamma broadcasts; then rope.
            def _mk_deferred(pso_tile):
                u = []

                def qkv1(oc, lq):
                    ps = pso_tile([128, 512], F32)
                    for cc in range(8):
                        nc.tensor.matmul(
                            ps[:, :],
                            wq_sb[cc][:, oc * 128:(oc + 1) * 128],
                            xt_sb[cc][lq][:, :],
                            start=(cc == 0), stop=(cc == 7),
                        )
                    nc.vector.tensor_scalar_add(
                        chunks[oc][:, lq * 512:(lq + 1) * 512],
                        ps[:, :], bias_t[:, oc:oc + 1])
                for oc in (3, 4, 5):
                    for lq in range(4):
                        u.append((1700, lambda oc=oc, lq=lq: qkv1(oc, lq)))

                u.append((0, lambda: nc.vector.tensor_mul(rA[:, :], E1[:, :], E1[:, :])))
                u.append((0, lambda: nc.vector.tensor_mul(rB[:, :], O1[:, :], O1[:, :])))

                def stats_ln():
                    # one item: all four Ln calls back-to-back (1 table load)
                    for lw in range(4):
                        ps4 = pso_tile([4, 512], F32)
                        nc.tensor.matmul(ps4[:, :], ind_t[:, :],
                                         rA[:, lw * 512:(lw + 1) * 512],
                                         start=True, stop=False)
                        nc.tensor.matmul(ps4[:, :], ind_t[:, :],
                                         rB[:, lw * 512:(lw + 1) * 512],
                                         start=False, stop=True)
                        nc.scalar.activation(lnv1[lw][:, :], ps4[:, :], AF.Ln,
                                             scale=1.0 / 64.0,
                                             bias=eps_t[:, 0:1])

                def stats_exp():
                    for lw in range(4):
                        nc.scalar.activation(iv1[lw][:, :], lnv1[lw][:, :],
                                             AF.Exp, scale=-0.5,
                                             bias=lnb_t[:, 0:1])

                def msb(kind, lw):
                    gsl = gind_t[:, (2 + kind) * 128:(2 + kind + 1) * 128]
                    mm = pso_tile([128, 512], F32)
                    nc.tensor.matmul(mm[:, :], gsl, iv1[lw][:, :],
                                     start=True, stop=True)
                    nc.vector.tensor_copy(
                        M_sb[kind][:, lw * 512:(lw + 1) * 512], mm[:, :])

                def vtr(lw):
                    pt = pso_tile([128, 128], BF16)
                    nc.tensor.transpose(pt[:, :], V1[:, lw * 128:(lw + 1) * 128],
                                        ident[:, :])
                    vv = vseq[1][lw].rearrange("a (h x) -> a h x", h=2)
                    nc.vector.tensor_copy(
                        vv[:, :, 0:64],
                        pt[:, :].rearrange("a (h x) -> a h x", h=2))
                    nc.vector.tensor_copy(vv[:, :, 64], onecol[:, :])

                for lw in range(10):
                    u.append((100, lambda lw=lw: vtr(lw)))
                u.append((900, stats_ln))
                for lw in range(10, 13):
                    u.append((100, lambda lw=lw: vtr(lw)))
                u.append((0, stats_exp))
                for lw in range(13, NKV):
                    u.append((100, lambda lw=lw: vtr(lw)))
                for kind in range(2):
                    for lw in range(4):
                        u.append((250, lambda k=kind, lw=lw: msb(k, lw)))

                u.append((0, lambda: nc.vector.tensor_mul(rA[:, :], E1[:, :], cos_t[:, :])))
                u.append((0, lambda: nc.vector.tensor_mul(rB[:, :], O1[:, :], sin_t[:, :])))
                u.append((0, lambda: nc.vector.tensor_sub(rA[:, :], rA[:, :], rB[:, :])))
                u.append((0, lambda: nc.vector.tensor_mul(sE1[:, :], rA[:, :], M_sb[0][:, :])))
                u.append((0, lambda: nc.vector.tensor_mul(rA[:, :], E1[:, :], sin_t[:, :])))
                u.append((0, lambda: nc.vector.tensor_mul(rB[:, :], O1[:, :], cos_t[:, :])))
                u.append((0, lambda: nc.vector.tensor_add(rA[:, :], rA[:, :], rB[:, :])))
                u.append((0, lambda: nc.vector.tensor_mul(sO1[:, :], rA[:, :], M_sb[1][:, :])))
                for blk in range(2):
                    for args in (
                        (qT[1][blk * 64:blk * 64 + 32, :], sE1[blk * 32:(blk + 1) * 32, :]),
                        (qT[1][blk * 64 + 32:blk * 64 + 64, :], sO1[blk * 32:(blk + 1) * 32, :]),
                        (kT[1][blk * 64:blk * 64 + 32, :], sE1[64 + blk * 32:64 + (blk + 1) * 32, :]),
                        (kT[1][blk * 64 + 32:blk * 64 + 64, :], sO1[64 + blk * 32:64 + (blk + 1) * 32, :]),
                    ):
                        u.append((0, lambda a=args: nc.sync.dma_start(out=a[0], in_=a[1])))
                u.append((0, lambda: chkD.release()))
                u.append((0, lambda: xw.release()))
                return u

            # ---- phase 3: attention + fused output projection ----
            # Per (pair, q-block of 512): flash loop over 16 kv chunks.
            # Emission is software-pipelined (scores[j+1] ahead of PV[j]) so
            # the PE stream never head-blocks on the EXP; the out-projection
            # for q-block qb is dripped into the following windows as PE
            # filler (keeps the HAM clock warm).
            with (
                tc.tile_pool(name="exp", bufs=4) as xp,
                tc.tile_pool(name="den", bufs=2) as dn,
                tc.tile_pool(name="oev", bufs=2) as oe,
                tc.tile_pool(name="psa", bufs=2, space="PSUM") as psa,
                tc.tile_pool(name="psv", bufs=1, space="PSUM") as psv,
                tc.tile_pool(name="pso", bufs=2, space="PSUM") as pso,
            ):
                _poctr = [0]

                def _po_tile(shape, dt):
                    _poctr[0] += 1
                    return pso.tile(shape, dt, tag="po",
                                    name=f"pof{_poctr[0]}")

                filler = _mk_deferred(_po_tile)

                # out-projection split across pairs: pair0's half runs as
                # filler during its own attention (PSUM -> bf16 SBUF
                # partials), pair1's half adds them back. Both phases get a
                # real matmul per window, which keeps the PE HAM clock warm.
                def mk_proj0(qb):
                    def one(i, lw, n):
                        def run():
                            po = _po_tile([128, 512], F32)
                            nc.tensor.matmul(
                                po[:, :],
                                oq[0][qb][:, lw * 128:(lw + 1) * 128],
                                wo_t[0][:, n * 512:(n + 1) * 512],
                                start=True, stop=True)
                            nc.vector.tensor_copy(pA[qb][i][:, :], po[:, :])
                        return run
                    return [(450, one(i, lw, n))
                            for i, (lw, n) in enumerate(
                                (lw, n) for lw in range(4) for n in range(2))]

                def mk_proj1(qb):
                    def one(i, lw, n):
                        def run():
                            po = _po_tile([128, 512], F32)
                            nc.tensor.matmul(
                                po[:, :],
                                oq[1][qb][:, lw * 128:(lw + 1) * 128],
                                wo_t[1][:, n * 512:(n + 1) * 512],
                                start=True, stop=True)
                            ov = oe.tile([128, 512], F16, tag="ov")
                            nc.vector.tensor_add(ov[:, :], po[:, :],
                                                 pA[qb][i][:, :])
                            nc.sync.dma_start(
                                out=out_d[qb * LQB + lw * 128:
                                          qb * LQB + (lw + 1) * 128,
                                          n * 512:(n + 1) * 512],
                                in_=ov[:, :])
                        return run
                    return [(450, one(i, lw, n))
                            for i, (lw, n) in enumerate(
                                (lw, n) for lw in range(4) for n in range(2))]

                for p in range(NPAIR):
                    if p == 1:
                        while filler:
                            filler.pop(0)[1]()
                    for qb in range(NQB):
                        q0 = qb * LQB
                        oA = psv.tile([65, LQB], F32, tag="oA")
                        oB = psv.tile([65, LQB], F32, tag="oB")

                        def emit_scores(j):
                            sAB = psa.tile([128, 2 * LQB], F32, tag="sAB")
                            nc.tensor.matmul(
                                sAB[:, 0:LQB],
                                kT[p][0:64, j * 128:(j + 1) * 128],
                                qT[p][0:64, q0:q0 + LQB],
                                start=True, stop=True, tile_position=(0, 0))
                            nc.tensor.matmul(
                                sAB[:, LQB:2 * LQB],
                                kT[p][64:128, j * 128:(j + 1) * 128],
                                qT[p][64:128, q0:q0 + LQB],
                                start=True, stop=True, tile_position=(64, 0))
                            return sAB

                        def emit_pv(j, eAB):
                            nc.tensor.matmul(
                                oA[:, :], vseq[p][j][:, 0:65],
                                eAB[:, 0:LQB],
                                start=(j == 0), stop=(j == NKV - 1))
                            nc.tensor.matmul(
                                oB[:, :], vseq[p][j][:, 65:130],
                                eAB[:, LQB:2 * LQB],
                                start=(j == 0), stop=(j == NKV - 1))

                        prev_e = None
                        for j in range(NKV):
                            sAB = emit_scores(j)
                            if prev_e is not None:
                                emit_pv(j - 1, prev_e)
                            eAB = xp.tile([128, 2 * LQB], BF16, tag="eAB")
                            nc.scalar.activation(eAB[:, :], sAB[:, :], AF.Exp)
                            prev_e = eAB
                            got = 0
                            if filler:
                                w, fn = filler.pop(0)
                                fn()
                                got += w
                            if len(filler) > 28:
                                w, fn = filler.pop(0)
                                fn()
                                got += w
                        emit_pv(NKV - 1, prev_e)

                        # normalize: o / denom (denom = row 64); the
                        # ones-row matmul broadcast beats a DMA+GpSimd route
                        # (the DMA round-trip stalls each q-block boundary)
                        for hd, oo in enumerate((oA, oB)):
                            den = dn.tile([65, LQB], F32R, tag="den")
                            nc.vector.tensor_copy(den[64:65, :], oo[64:65, :])
                            dbc = pso.tile([64, LQB], F32, tag="po")
                            nc.tensor.matmul(
                                dbc[:, :], ones_t[64:65, :],
                                den[64:65, :], start=True, stop=True)
                            rcb = dn.tile([64, LQB], F32, tag="rcb")
                            nc.vector.reciprocal_approx_fast(rcb[:, :], dbc[:, :])
                            onrm = dn.tile([64, LQB], BF16, tag="onrm")
                            nc.vector.tensor_mul(onrm[:, :], oo[0:64, :], rcb[:, :])
                            nc.sync.dma_start(
                                out=oq[p][qb][hd * 64:(hd + 1) * 64, :],
                                in_=onrm[:, :])
                        filler.extend(
                            (mk_proj0 if p == 0 else mk_proj1)(qb))
                while filler:
                    filler.pop(0)[1]()


            dfr2.release()
            pav.release()
            dfr.release()
            lv.release()
    nc.compile()
    return nc


_PROG = None


def _get_program():
    global _PROG
    if _PROG is None:
        _PROG = _build_program()
    return _PROG


def _core_inputs(core, x, w_qkv, b_qkv, w_out, q_gamma, k_gamma,
                 cos_tab, sin_tab, ind, ones128):
    b = core // 4
    hb = (core % 4) * 4
    # row permutation of w_qkv for this core
    perm = []
    for p in range(NPAIR):
        hA, hB = hb + 2 * p, hb + 2 * p + 1
        for h in (hA, hB):                      # E chunk: q evens, k evens
            perm += [h * 64 + 2 * j for j in range(32)]
        for h in (hA, hB):
            perm += [1024 + h * 64 + 2 * j for j in range(32)]
        for h in (hA, hB):                      # O chunk
            perm += [h * 64 + 2 * j + 1 for j in range(32)]
        for h in (hA, hB):
            perm += [1024 + h * 64 + 2 * j + 1 for j in range(32)]
        for h in (hA, hB):                      # V chunk
            perm += [2048 + h * 64 + c for c in range(64)]
    perm = np.asarray(perm)
    w_local = w_qkv[perm]                       # [768, 1024]
    wq = np.ascontiguousarray(w_local.T)        # [1024, 768]
    bias6 = np.ascontiguousarray(b_qkv[perm].reshape(6, 128))

    # gamma-scaled indicator lhsT [4, 4*128]: (p, kind) -> [4, 128]
    gind = np.zeros((4, 4 * 128), np.float32)
    for p in range(NPAIR):
        for kind in range(2):                   # 0=E(evens), 1=O(odds)
            blk = (2 * p + kind) * 128
            for r in range(128):
                g = r // 32
                h = hb + 2 * p + (g % 2)
                ch = 2 * (r % 32) + kind
                gam = q_gamma[h, ch] if g < 2 else k_gamma[h, ch]
                gind[g, blk + r] = gam

    # w_out slice: [256, 1024]
    wo = np.empty((256, 1024), np.float32)
    for p in range(NPAIR):
        for i, h in enumerate((hb + 2 * p, hb + 2 * p + 1)):
            wo[p * 128 + i * 64:p * 128 + (i + 1) * 64, :] = \
                w_out[:, h * 64:(h + 1) * 64].T

    return {
        "xt": np.ascontiguousarray(x[b].T).astype(ml_dtypes.bfloat16),
        "wq": wq.astype(ml_dtypes.bfloat16),
        "bias6": bias6,
        "cost": cos_tab,
        "sint": sin_tab,
        "ind": ind.astype(ml_dtypes.bfloat16),
        "gind": gind,
        "ones128": ones128,
        "wo": wo.astype(ml_dtypes.bfloat16),
    }


def kernel(x, w_qkv, b_qkv, w_out, b_out, q_gamma, k_gamma, _trace=False):
    x = np.asarray(x, np.float32)
    w_qkv = np.asarray(w_qkv, np.float32)
    b_qkv = np.asarray(b_qkv, np.float32)
    w_out = np.asarray(w_out, np.float32)
    b_out = np.asarray(b_out, np.float32)
    q_gamma = np.asarray(q_gamma, np.float32)
    k_gamma = np.asarray(k_gamma, np.float32)

    inv_freq = (1.0 / ROPE_THETA ** (np.arange(32, dtype=np.float64) / 32.0))
    ang = np.arange(L, dtype=np.float64)[None, :] * \
        np.tile(inv_freq, 4)[:, None]          # [128, L], row r -> freq r%32
    cos_tab = np.cos(ang).astype(ml_dtypes.bfloat16)
    sin_tab = np.sin(ang).astype(ml_dtypes.bfloat16)
    ind = np.zeros((128, 4), np.float32)
    for r in range(128):
        ind[r, r // 32] = 1.0
    ones128 = np.ones((128, 64), np.float32)

    nc = _get_program()
    in_maps = [_core_inputs(c, x, w_qkv, b_qkv, w_out, q_gamma, k_gamma,
                            cos_tab, sin_tab, ind, ones128)
               for c in range(NCORES)]
    r = run_bass_kernel_spmd(nc, in_maps, list(range(NCORES)), trace=_trace)
    out = np.zeros((B, L, C), np.float32)
    for c in range(NCORES):
        out[c // 4] += r.results[c]["out"].astype(np.float32)
    out += b_out[None, None, :]
    if _trace:
        kernel._last_results = r
    return out



# revision 51
# speedup vs baseline: 1.1057x; 1.0202x over previous
"""MultiHeadAttention (RoPE + QK-RMSNorm, non-causal) on 8 trn2 NeuronCores.

Sharding: batch (2) x head-groups (4 heads each) -> 8 cores. Each core:
  - QKV projection for its 4 heads (768 output channels) from x[b] (full seq)
  - RoPE + QK-RMSNorm fused via channel permutation + per-row scale matmuls
  - full 2048x2048 attention for 4 heads (flash-style, scores transposed,
    softmax without max-subtraction: RMS-normed logits are bounded by 8)
  - output projection partial (its 256 channels of the 1024-ch contraction)
Host: sums the 4 partials per batch (fp16 device outputs) and adds b_out.

Performance structure (tuned against perfetto + HAM traces):
  - every matmul operand is bf16 (fp16 streams at HALF the PE rate on TRN2;
    bf16/f32r run the full 2.4 GHz column rate)
  - the attention j-loop is ScalarE-bound (EXP at (N+352)/1.2 ns); emission
    is software-pipelined (scores[j+1] before PV[j]) and everything movable
    (pair1 QKV chains, its RMS stats / gamma broadcasts / V transposes /
    rope, and the split output projection) drips into the windows as PE
    filler so the PE HAM clock stays at 8/8
  - Ln and Exp live in different ACT table sets (~1.3us reload per switch),
    so the RMS-norm batches all Lns, then all Exps
  - the output projection is split per pair: pair0's half accumulates to
    bf16 SBUF partials during its own attention, pair1 adds them back
"""
import math
import ml_dtypes
import numpy as np

import concourse.bass as bass
from concourse import bacc
import concourse.mybir as mybir
import concourse.tile as tile
from concourse.bass_utils import run_bass_kernel_spmd
from concourse.masks import make_identity

F32 = mybir.dt.float32
F32R = mybir.dt.float32r
F16 = mybir.dt.float16
BF16 = mybir.dt.bfloat16
AF = mybir.ActivationFunctionType

B, L, C, H, D = 2, 2048, 1024, 16, 64
NCORES = 8
ROPE_THETA = 10000.0
RMS_EPS = 1e-6
NPAIR = 2        # head pairs per core
LQB = 512        # q block size
NQB = L // LQB   # 4 q blocks
NKV = L // 128   # 16 kv chunks


def _build_program():
    nc = bacc.Bacc("TRN2", target_bir_lowering=False, debug=False)

    xt_d = nc.dram_tensor("xt", [C, L], BF16, kind="ExternalInput")
    wq_d = nc.dram_tensor("wq", [C, 768], BF16, kind="ExternalInput")
    bias_d = nc.dram_tensor("bias6", [6, 128], F32, kind="ExternalInput")
    cos_d = nc.dram_tensor("cost", [128, L], BF16, kind="ExternalInput")
    sin_d = nc.dram_tensor("sint", [128, L], BF16, kind="ExternalInput")
    ind_d = nc.dram_tensor("ind", [128, 4], BF16, kind="ExternalInput")
    gind_d = nc.dram_tensor("gind", [4, 4 * 128], F32R, kind="ExternalInput")
    ones_d = nc.dram_tensor("ones128", [128, 64], F32R, kind="ExternalInput")
    wo_d = nc.dram_tensor("wo", [256, 1024], BF16, kind="ExternalInput")
    out_d = nc.dram_tensor("out", [L, C], F16, kind="ExternalOutput")

    with tile.TileContext(nc) as tc:
        with tc.tile_pool(name="const", bufs=1) as cp:
            # ---- constant tiles (DMAs for late-use consts issued after the
            # hot-path wq/xt loads so the first QKV chains start ASAP) ----
            cos_t = cp.tile([128, L], BF16, tag="cos")
            sin_t = cp.tile([128, L], BF16, tag="sin")
            ind_t = cp.tile([128, 4], BF16, tag="ind")
            gind_t = cp.tile([4, 4 * 128], F32R, tag="gind")
            ones_t = cp.tile([128, 64], F32R, tag="ones")
            wo_t = [cp.tile([128, 1024], BF16, tag=f"wo{p}", name=f"wo{p}")
                    for p in range(2)]
            bias_t = cp.tile([128, 6], F32, tag="bias")
            lnb_t = cp.tile([4, 1], F32, tag="lnb")
            nc.vector.memset(lnb_t[:, :], 0.0)
            nc.vector.memset(lnb_t[0:2, :], -math.log(8.0))
            eps_t = cp.tile([4, 1], F32, tag="eps")
            nc.vector.memset(eps_t[:, :], RMS_EPS)
            ident = cp.tile([128, 128], BF16, tag="ident")
            make_identity(nc, ident[:, :])
            onecol = cp.tile([128, 2], F32, tag="onecol")
            nc.vector.memset(onecol[:, :], 1.0)

            # xw below the chunk pools on the right stack: released last
            # (from inside the attention filler, after chkD)
            xw = tc.alloc_tile_pool(name="xw", bufs=1, side="right")
            # E/O/V chunks per pair: rows of E = [qA_e, qB_e, kA_e, kB_e]
            chkD = tc.alloc_tile_pool(name="chkD", bufs=1, side="right")
            chk = tc.alloc_tile_pool(name="chk", bufs=1, side="right")
            chunks = [
                (chkD if i in (3, 4, 5) else chk).tile(
                    [128, L], BF16,
                    tag=f"c{i}", name=f"c{i}") for i in range(6)]

            # ---- phase 1: fused QKV projection (pair0 chunks up front;
            # pair1's chains run as attention filler, xw stays alive) ----
            with tc.tile_pool(name="psq", bufs=2, space="PSUM") as psq:
                wq_sb = []
                for cc in range(8):
                    wqi = xw.tile([128, 768], BF16, tag=f"w{cc}", name=f"w{cc}")
                    nc.sync.dma_start(out=wqi, in_=wq_d[cc * 128:(cc + 1) * 128, :])
                    wq_sb.append(wqi)
                nc.sync.dma_start(out=bias_t, in_=bias_d[:, :].transpose([1, 0]))
                xt_sb = [[None] * 4 for _ in range(8)]
                for lq in range(4):
                    for cc in range(8):
                        xti = xw.tile([128, 512], BF16, tag=f"x{cc}_{lq}",
                                      name=f"x{cc}_{lq}")
                        nc.sync.dma_start(
                            out=xti,
                            in_=xt_d[cc * 128:(cc + 1) * 128,
                                     lq * 512:(lq + 1) * 512])
                        xt_sb[cc][lq] = xti
                # late-use constants after the hot path
                nc.sync.dma_start(out=ind_t, in_=ind_d[:, :])
                nc.sync.dma_start(out=cos_t, in_=cos_d[:, :])
                nc.sync.dma_start(out=sin_t, in_=sin_d[:, :])
                nc.sync.dma_start(out=gind_t, in_=gind_d[:, :])
                nc.sync.dma_start(out=ones_t, in_=ones_d[:, :])
                for p in range(2):
                    nc.sync.dma_start(out=wo_t[p],
                                      in_=wo_d[p * 128:(p + 1) * 128, :])
                for lq in range(4):
                    for oc in range(3):
                        ps = psq.tile([128, 512], F32, tag="ps")
                        for cc in range(8):
                            nc.tensor.matmul(
                                ps[:, :],
                                wq_sb[cc][:, oc * 128:(oc + 1) * 128],
                                xt_sb[cc][lq][:, :],
                                start=(cc == 0), stop=(cc == 7),
                            )
                        nc.vector.tensor_scalar_add(
                            chunks[oc][:, lq * 512:(lq + 1) * 512],
                            ps[:, :], bias_t[:, oc:oc + 1])

            # long-lived attention operands (allocated after xw released)
            lv = tc.alloc_tile_pool(name="live", bufs=1)
            qT, kT, vseq = [], [], []
            for p in range(NPAIR):
                qT.append(lv.tile([128, L], BF16, tag=f"qT{p}", name=f"qT{p}"))
                kT.append(lv.tile([128, L], BF16, tag=f"kT{p}", name=f"kT{p}"))
                vseq.append([lv.tile([128, 130], BF16, tag=f"vs{p}_{lw}",
                                     name=f"vs{p}_{lw}") for lw in range(NKV)])
            # normalized attention outputs, per (pair, q-block)
            oq = [[lv.tile([128, LQB], BF16, tag=f"oq{p}_{qb}",
                           name=f"oq{p}_{qb}") for qb in range(NQB)]
                  for p in range(NPAIR)]

            # ---- phase 2: pair0 rope + rmsnorm + relocation + v transpose
            # (pair1's prep is deferred into pair0's attention as filler) ----
            dfr = tc.alloc_tile_pool(name="dfr", bufs=1)
            M_sb = [dfr.tile([128, L], BF16, tag=f"Msb{k}", name=f"Msb{k}")
                    for k in range(2)]
            with (
                tc.tile_pool(name="tmp", bufs=1) as tp,
                tc.tile_pool(name="psp", bufs=2, space="PSUM") as psp,
            ):
                E, O, V = chunks[0], chunks[1], chunks[2]
                sqE = tp.tile([128, L], BF16, tag="bigA")
                nc.vector.tensor_mul(sqE[:, :], E[:, :], E[:, :])
                sqO = tp.tile([128, L], BF16, tag="bigB")
                nc.vector.tensor_mul(sqO[:, :], O[:, :], O[:, :])
                # batch all Ln calls, then all Exp calls: Ln and Exp live in
                # different ACT table sets, and every alternation costs a
                # ~1.3us ACT_TABLE_LOAD
                lnvs = []
                for lw in range(4):
                    ps4 = psp.tile([4, 512], F32, tag="ps4")
                    nc.tensor.matmul(ps4[:, :], ind_t[:, :],
                                     sqE[:, lw * 512:(lw + 1) * 512],
                                     start=True, stop=False)
                    nc.tensor.matmul(ps4[:, :], ind_t[:, :],
                                     sqO[:, lw * 512:(lw + 1) * 512],
                                     start=False, stop=True)
                    lnv = tp.tile([4, 512], F32, tag=f"lnv{lw}",
                                  name=f"lnv{lw}")
                    nc.scalar.activation(lnv[:, :], ps4[:, :], AF.Ln,
                                         scale=1.0 / 64.0, bias=eps_t[:, 0:1])
                    lnvs.append(lnv)
                invrs = []
                for lw in range(4):
                    iv = tp.tile([4, 512], F32R, tag=f"inv{lw}",
                                 name=f"inv{lw}")
                    nc.scalar.activation(iv[:, :], lnvs[lw][:, :], AF.Exp,
                                         scale=-0.5, bias=lnb_t[:, 0:1])
                    invrs.append(iv)
                # rope now (bf16, 3 temps: A, B, C)
                t1c = tp.tile([128, L], BF16, tag="bigC")
                nc.vector.tensor_mul(t1c[:, :], E[:, :], cos_t[:, :])
                t2s = tp.tile([128, L], BF16, tag="bigB")
                nc.vector.tensor_mul(t2s[:, :], O[:, :], sin_t[:, :])
                rE = tp.tile([128, L], BF16, tag="bigA")
                nc.vector.tensor_sub(rE[:, :], t1c[:, :], t2s[:, :])
                t1s = tp.tile([128, L], BF16, tag="bigC")
                nc.vector.tensor_mul(t1s[:, :], E[:, :], sin_t[:, :])
                t2c = tp.tile([128, L], BF16, tag="bigB")
                nc.vector.tensor_mul(t2c[:, :], O[:, :], cos_t[:, :])
                rO = t1s
                nc.vector.tensor_add(rO[:, :], t1s[:, :], t2c[:, :])
                sE = tp.tile([128, L], BF16, tag="sc16E")
                sO = tp.tile([128, L], BF16, tag="sc16O")
                for kind, (rt, st) in enumerate(((rE, sE), (rO, sO))):
                    gsl = gind_t[:, kind * 128:(kind + 1) * 128]
                    for lw in range(4):
                        mm = psp.tile([128, 512], F32, tag="mps")
                        nc.tensor.matmul(mm[:, :], gsl,
                                         invrs[lw][:, :],
                                         start=True, stop=True)
                        nc.vector.tensor_mul(
                            st[:, lw * 512:(lw + 1) * 512],
                            rt[:, lw * 512:(lw + 1) * 512], mm[:, :])
                for blk in range(2):
                    nc.sync.dma_start(out=qT[0][blk * 64:blk * 64 + 32, :],
                                      in_=sE[blk * 32:(blk + 1) * 32, :])
                    nc.sync.dma_start(out=qT[0][blk * 64 + 32:blk * 64 + 64, :],
                                      in_=sO[blk * 32:(blk + 1) * 32, :])
                    nc.sync.dma_start(out=kT[0][blk * 64:blk * 64 + 32, :],
                                      in_=sE[64 + blk * 32:64 + (blk + 1) * 32, :])
                    nc.sync.dma_start(out=kT[0][blk * 64 + 32:blk * 64 + 64, :],
                                      in_=sO[64 + blk * 32:64 + (blk + 1) * 32, :])
                # v transpose -> vseq [l,130]: [vA(64) 1 vB(64) 1]
                for lw in range(NKV):
                    pt = psp.tile([128, 128], BF16, tag="ptr")
                    nc.tensor.transpose(pt[:, :],
                                        V[:, lw * 128:(lw + 1) * 128],
                                        ident[:, :])
                    vv = vseq[0][lw].rearrange("a (h x) -> a h x", h=2)
                    nc.vector.tensor_copy(
                        vv[:, :, 0:64],
                        pt[:, :].rearrange("a (h x) -> a h x", h=2))
                    nc.vector.tensor_copy(vv[:, :, 64], onecol[:, :])

            chk.release()
            # pair0 half of the output projection, bf16 partials
            pav = tc.alloc_tile_pool(name="pav", bufs=1)
            pA = [[pav.tile([128, 512], BF16, tag=f"pA{qb}_{i}",
                            name=f"pA{qb}_{i}") for i in range(8)]
                  for qb in range(NQB)]
            dfr2 = tc.alloc_tile_pool(name="dfr2", bufs=1)
            rA = dfr2.tile([128, L], BF16, tag="rA")
            rB = dfr2.tile([128, L], BF16, tag="rB")
            sE1 = dfr2.tile([128, L], BF16, tag="sE1")
            sO1 = dfr2.tile([128, L], BF16, tag="sO1")
            lnv1 = [dfr2.tile([4, 512], F32, tag=f"lnv1_{lw}",
                              name=f"lnv1_{lw}") for lw in range(4)]
            iv1 = [dfr2.tile([4, 512], F32R, tag=f"iv1_{lw}",
                             name=f"iv1_{lw}") for lw in range(4)]
            E1, O1, V1 = chunks[3], chunks[4], chunks[5]

            # pair1 prep, dripped one item per attention window of pair0.
            # PSUM scratch comes from the attention-phase "po" slots, so the
            # closures must run inside the attention pool scope.
            # Filler ordering matters: engines execute in program order, so a
            # filler op whose producer (on another engine) hasn't run yet
            # head-blocks every later op on its engine — including the EXP
            # stream. Squares go first; ACT-free transpose items pad the gap
            # until the squares have certainly retired; only then the
            # Ln/Exp stats; then the gamma broadcasts; then rope.
            def _mk_deferred(pso_tile):
                u = []

                def qkv1(oc, lq):
                    ps = pso_tile([128, 512], F32)
                    for cc in range(8):
                        nc.tensor.matmul(
                            ps[:, :],
                            wq_sb[cc][:, oc * 128:(oc + 1) * 128],
                            xt_sb[cc][lq][:, :],
                            start=(cc == 0), stop=(cc == 7),
                        )
                    nc.vector.tensor_scalar_add(
                        chunks[oc][:, lq * 512:(lq + 1) * 512],
                        ps[:, :], bias_t[:, oc:oc + 1])
                for oc in (3, 4, 5):
                    for lq in range(4):
                        u.append((1700, lambda oc=oc, lq=lq: qkv1(oc, lq)))

                u.append((0, lambda: nc.vector.tensor_mul(rA[:, :], E1[:, :], E1[:, :])))
                u.append((0, lambda: nc.vector.tensor_mul(rB[:, :], O1[:, :], O1[:, :])))

                def stats_ln():
                    # one item: all four Ln calls back-to-back (1 table load)
                    for lw in range(4):
                        ps4 = pso_tile([4, 512], F32)
                        nc.tensor.matmul(ps4[:, :], ind_t[:, :],
                                         rA[:, lw * 512:(lw + 1) * 512],
                                         start=True, stop=False)
                        nc.tensor.matmul(ps4[:, :], ind_t[:, :],
                                         rB[:, lw * 512:(lw + 1) * 512],
                                         start=False, stop=True)
                        nc.scalar.activation(lnv1[lw][:, :], ps4[:, :], AF.Ln,
                                             scale=1.0 / 64.0,
                                             bias=eps_t[:, 0:1])

                def stats_exp():
                    for lw in range(4):
                        nc.scalar.activation(iv1[lw][:, :], lnv1[lw][:, :],
                                             AF.Exp, scale=-0.5,
                                             bias=lnb_t[:, 0:1])

                def msb(kind, lw):
                    gsl = gind_t[:, (2 + kind) * 128:(2 + kind + 1) * 128]
                    mm = pso_tile([128, 512], F32)
                    nc.tensor.matmul(mm[:, :], gsl, iv1[lw][:, :],
                                     start=True, stop=True)
                    nc.vector.tensor_copy(
                        M_sb[kind][:, lw * 512:(lw + 1) * 512], mm[:, :])

                def vtr(lw):
                    pt = pso_tile([128, 128], BF16)
                    nc.tensor.transpose(pt[:, :], V1[:, lw * 128:(lw + 1) * 128],
                                        ident[:, :])
                    vv = vseq[1][lw].rearrange("a (h x) -> a h x", h=2)
                    nc.vector.tensor_copy(
                        vv[:, :, 0:64],
                        pt[:, :].rearrange("a (h x) -> a h x", h=2))
                    nc.vector.tensor_copy(vv[:, :, 64], onecol[:, :])

                for lw in range(10):
                    u.append((100, lambda lw=lw: vtr(lw)))
                u.append((900, stats_ln))
                for lw in range(10, 13):
                    u.append((100, lambda lw=lw: vtr(lw)))
                u.append((0, stats_exp))
                for lw in range(13, NKV):
                    u.append((100, lambda lw=lw: vtr(lw)))
                for kind in range(2):
                    for lw in range(4):
                        u.append((250, lambda k=kind, lw=lw: msb(k, lw)))

                u.append((0, lambda: nc.vector.tensor_mul(rA[:, :], E1[:, :], cos_t[:, :])))
                u.append((0, lambda: nc.vector.tensor_mul(rB[:, :], O1[:, :], sin_t[:, :])))
                u.append((0, lambda: nc.vector.tensor_sub(rA[:, :], rA[:, :], rB[:, :])))
                u.append((0, lambda: nc.vector.tensor_mul(sE1[:, :], rA[:, :], M_sb[0][:, :])))
                u.append((0, lambda: nc.vector.tensor_mul(rA[:, :], E1[:, :], sin_t[:, :])))
                u.append((0, lambda: nc.vector.tensor_mul(rB[:, :], O1[:, :], cos_t[:, :])))
                u.append((0, lambda: nc.vector.tensor_add(rA[:, :], rA[:, :], rB[:, :])))
                u.append((0, lambda: nc.vector.tensor_mul(sO1[:, :], rA[:, :], M_sb[1][:, :])))
                for blk in range(2):
                    for args in (
                        (qT[1][blk * 64:blk * 64 + 32, :], sE1[blk * 32:(blk + 1) * 32, :]),
                        (qT[1][blk * 64 + 32:blk * 64 + 64, :], sO1[blk * 32:(blk + 1) * 32, :]),
                        (kT[1][blk * 64:blk * 64 + 32, :], sE1[64 + blk * 32:64 + (blk + 1) * 32, :]),
                        (kT[1][blk * 64 + 32:blk * 64 + 64, :], sO1[64 + blk * 32:64 + (blk + 1) * 32, :]),
                    ):
                        u.append((0, lambda a=args: nc.sync.dma_start(out=a[0], in_=a[1])))
                u.append((0, lambda: chkD.release()))
                u.append((0, lambda: xw.release()))
                return u

            # ---- phase 3: attention + fused output projection ----
            # Per (pair, q-block of 512): flash loop over 16 kv chunks.
            # Emission is software-pipelined (scores[j+1] ahead of PV[j]) so
            # the PE stream never head-blocks on the EXP; the out-projection
            # for q-block qb is dripped into the following windows as PE
            # filler (keeps the HAM clock warm).
            with (
                tc.tile_pool(name="exp", bufs=6) as xp,
                tc.tile_pool(name="den", bufs=3) as dn,
                tc.tile_pool(name="oev", bufs=2) as oe,
                tc.tile_pool(name="psa", bufs=2, space="PSUM") as psa,
                tc.tile_pool(name="psv", bufs=1, space="PSUM") as psv,
                tc.tile_pool(name="pso", bufs=2, space="PSUM") as pso,
            ):
                _poctr = [0]

                def _po_tile(shape, dt):
                    _poctr[0] += 1
                    return pso.tile(shape, dt, tag="po",
                                    name=f"pof{_poctr[0]}")

                filler = _mk_deferred(_po_tile)

                # out-projection split across pairs: pair0's half runs as
                # filler during its own attention (PSUM -> bf16 SBUF
                # partials), pair1's half adds them back. Both phases get a
                # real matmul per window, which keeps the PE HAM clock warm.
                def mk_proj0(qb):
                    def one(i, lw, n):
                        def run():
                            po = _po_tile([128, 512], F32)
                            nc.tensor.matmul(
                                po[:, :],
                                oq[0][qb][:, lw * 128:(lw + 1) * 128],
                                wo_t[0][:, n * 512:(n + 1) * 512],
                                start=True, stop=True)
                            nc.vector.tensor_copy(pA[qb][i][:, :], po[:, :])
                        return run
                    return [(450, one(i, lw, n))
                            for i, (lw, n) in enumerate(
                                (lw, n) for lw in range(4) for n in range(2))]

                def mk_proj1(qb):
                    def one(i, lw, n):
                        def run():
                            po = _po_tile([128, 512], F32)
                            nc.tensor.matmul(
                                po[:, :],
                                oq[1][qb][:, lw * 128:(lw + 1) * 128],
                                wo_t[1][:, n * 512:(n + 1) * 512],
                                start=True, stop=True)
                            ov = oe.tile([128, 512], F16, tag="ov")
                            nc.vector.tensor_add(ov[:, :], po[:, :],
                                                 pA[qb][i][:, :])
                            nc.sync.dma_start(
                                out=out_d[qb * LQB + lw * 128:
                                          qb * LQB + (lw + 1) * 128,
                                          n * 512:(n + 1) * 512],
                                in_=ov[:, :])
                        return run
                    return [(450, one(i, lw, n))
                            for i, (lw, n) in enumerate(
                                (lw, n) for lw in range(4) for n in range(2))]

                for p in range(NPAIR):
                    if p == 1:
                        while filler:
                            filler.pop(0)[1]()
                    for qb in range(NQB):
                        q0 = qb * LQB
                        oA = psv.tile([65, LQB], F32, tag="oA")
                        oB = psv.tile([65, LQB], F32, tag="oB")

                        def emit_scores(j):
                            sAB = psa.tile([128, 2 * LQB], F32, tag="sAB")
                            nc.tensor.matmul(
                                sAB[:, 0:LQB],
                                kT[p][0:64, j * 128:(j + 1) * 128],
                                qT[p][0:64, q0:q0 + LQB],
                                start=True, stop=True, tile_position=(0, 0))
                            nc.tensor.matmul(
                                sAB[:, LQB:2 * LQB],
                                kT[p][64:128, j * 128:(j + 1) * 128],
                                qT[p][64:128, q0:q0 + LQB],
                                start=True, stop=True, tile_position=(64, 0))
                            return sAB

                        def emit_pv(j, eAB):
                            nc.tensor.matmul(
                                oA[:, :], vseq[p][j][:, 0:65],
                                eAB[:, 0:LQB],
                                start=(j == 0), stop=(j == NKV - 1))
                            nc.tensor.matmul(
                                oB[:, :], vseq[p][j][:, 65:130],
                                eAB[:, LQB:2 * LQB],
                                start=(j == 0), stop=(j == NKV - 1))

                        prev_e = None
                        for j in range(NKV):
                            sAB = emit_scores(j)
                            if prev_e is not None:
                                emit_pv(j - 1, prev_e)
                            eAB = xp.tile([128, 2 * LQB], BF16, tag="eAB")
                            nc.scalar.activation(eAB[:, :], sAB[:, :], AF.Exp)
                            prev_e = eAB
                            got = 0
                            if filler:
                                w, fn = filler.pop(0)
                                fn()
                                got += w
                            if len(filler) > 24:
                                w, fn = filler.pop(0)
                                fn()
                                got += w
                        emit_pv(NKV - 1, prev_e)

                        # normalize: o / denom (denom = row 64); the
                        # ones-row matmul broadcast beats a DMA+GpSimd route
                        # (the DMA round-trip stalls each q-block boundary)
                        for hd, oo in enumerate((oA, oB)):
                            den = dn.tile([65, LQB], F32R, tag="den")
                            nc.vector.tensor_copy(den[64:65, :], oo[64:65, :])
                            dbc = pso.tile([64, LQB], F32, tag="po")
                            nc.tensor.matmul(
                                dbc[:, :], ones_t[64:65, :],
                                den[64:65, :], start=True, stop=True)
                            rcb = dn.tile([64, LQB], F32, tag="rcb")
                            nc.vector.reciprocal_approx_fast(rcb[:, :], dbc[:, :])
                            onrm = dn.tile([64, LQB], BF16, tag="onrm")
                            nc.vector.tensor_mul(onrm[:, :], oo[0:64, :], rcb[:, :])
                            nc.sync.dma_start(
                                out=oq[p][qb][hd * 64:(hd + 1) * 64, :],
                                in_=onrm[:, :])
                        filler.extend(
                            (mk_proj0 if p == 0 else mk_proj1)(qb))
                while filler:
                    filler.pop(0)[1]()


            dfr2.release()
            pav.release()
            dfr.release()
            lv.release()
    nc.compile()
    return nc


_PROG = None


def _get_program():
    global _PROG
    if _PROG is None:
        _PROG = _build_program()
    return _PROG


def _core_inputs(core, x, w_qkv, b_qkv, w_out, q_gamma, k_gamma,
                 cos_tab, sin_tab, ind, ones128):
    b = core // 4
    hb = (core % 4) * 4
    # row permutation of w_qkv for this core
    perm = []
    for p in range(NPAIR):
        hA, hB = hb + 2 * p, hb + 2 * p + 1
        for h in (hA, hB):                      # E chunk: q evens, k evens
            perm += [h * 64 + 2 * j for j in range(32)]
        for h in (hA, hB):
            perm += [1024 + h * 64 + 2 * j for j in range(32)]
        for h in (hA, hB):                      # O chunk
            perm += [h * 64 + 2 * j + 1 for j in range(32)]
        for h in (hA, hB):
            perm += [1024 + h * 64 + 2 * j + 1 for j in range(32)]
        for h in (hA, hB):                      # V chunk
            perm += [2048 + h * 64 + c for c in range(64)]
    perm = np.asarray(perm)
    w_local = w_qkv[perm]                       # [768, 1024]
    wq = np.ascontiguousarray(w_local.T)        # [1024, 768]
    bias6 = np.ascontiguousarray(b_qkv[perm].reshape(6, 128))

    # gamma-scaled indicator lhsT [4, 4*128]: (p, kind) -> [4, 128]
    gind = np.zeros((4, 4 * 128), np.float32)
    for p in range(NPAIR):
        for kind in range(2):                   # 0=E(evens), 1=O(odds)
            blk = (2 * p + kind) * 128
            for r in range(128):
                g = r // 32
                h = hb + 2 * p + (g % 2)
                ch = 2 * (r % 32) + kind
                gam = q_gamma[h, ch] if g < 2 else k_gamma[h, ch]
                gind[g, blk + r] = gam

    # w_out slice: [256, 1024]
    wo = np.empty((256, 1024), np.float32)
    for p in range(NPAIR):
        for i, h in enumerate((hb + 2 * p, hb + 2 * p + 1)):
            wo[p * 128 + i * 64:p * 128 + (i + 1) * 64, :] = \
                w_out[:, h * 64:(h + 1) * 64].T

    return {
        "xt": np.ascontiguousarray(x[b].T).astype(ml_dtypes.bfloat16),
        "wq": wq.astype(ml_dtypes.bfloat16),
        "bias6": bias6,
        "cost": cos_tab,
        "sint": sin_tab,
        "ind": ind.astype(ml_dtypes.bfloat16),
        "gind": gind,
        "ones128": ones128,
        "wo": wo.astype(ml_dtypes.bfloat16),
    }


def kernel(x, w_qkv, b_qkv, w_out, b_out, q_gamma, k_gamma, _trace=False):
    x = np.asarray(x, np.float32)
    w_qkv = np.asarray(w_qkv, np.float32)
    b_qkv = np.asarray(b_qkv, np.float32)
    w_out = np.asarray(w_out, np.float32)
    b_out = np.asarray(b_out, np.float32)
    q_gamma = np.asarray(q_gamma, np.float32)
    k_gamma = np.asarray(k_gamma, np.float32)

    inv_freq = (1.0 / ROPE_THETA ** (np.arange(32, dtype=np.float64) / 32.0))
    ang = np.arange(L, dtype=np.float64)[None, :] * \
        np.tile(inv_freq, 4)[:, None]          # [128, L], row r -> freq r%32
    cos_tab = np.cos(ang).astype(ml_dtypes.bfloat16)
    sin_tab = np.sin(ang).astype(ml_dtypes.bfloat16)
    ind = np.zeros((128, 4), np.float32)
    for r in range(128):
        ind[r, r // 32] = 1.0
    ones128 = np.ones((128, 64), np.float32)

    nc = _get_program()
    in_maps = [_core_inputs(c, x, w_qkv, b_qkv, w_out, q_gamma, k_gamma,
                            cos_tab, sin_tab, ind, ones128)
               for c in range(NCORES)]
    r = run_bass_kernel_spmd(nc, in_maps, list(range(NCORES)), trace=_trace)
    out = np.zeros((B, L, C), np.float32)
    for c in range(NCORES):
        out[c // 4] += r.results[c]["out"].astype(np.float32)
    out += b_out[None, None, :]
    if _trace:
        kernel._last_results = r
    return out



# revision 52
# speedup vs baseline: 1.1089x; 1.0030x over previous
"""MultiHeadAttention (RoPE + QK-RMSNorm, non-causal) on 8 trn2 NeuronCores.

Sharding: batch (2) x head-groups (4 heads each) -> 8 cores. Each core:
  - QKV projection for its 4 heads (768 output channels) from x[b] (full seq)
  - RoPE + QK-RMSNorm fused via channel permutation + per-row scale matmuls
  - full 2048x2048 attention for 4 heads (flash-style, scores transposed,
    softmax without max-subtraction: RMS-normed logits are bounded by 8)
  - output projection partial (its 256 channels of the 1024-ch contraction)
Host: sums the 4 partials per batch (fp16 device outputs) and adds b_out.

Performance structure (tuned against perfetto + HAM traces):
  - every matmul operand is bf16 (fp16 streams at HALF the PE rate on TRN2;
    bf16/f32r run the full 2.4 GHz column rate)
  - the attention j-loop is ScalarE-bound (EXP at (N+352)/1.2 ns); emission
    is software-pipelined (scores[j+1] before PV[j]) and everything movable
    (pair1 QKV chains, its RMS stats / gamma broadcasts / V transposes /
    rope, and the split output projection) drips into the windows as PE
    filler so the PE HAM clock stays at 8/8
  - Ln and Exp live in different ACT table sets (~1.3us reload per switch),
    so the RMS-norm batches all Lns, then all Exps
  - the output projection is split per pair: pair0's half accumulates to
    bf16 SBUF partials during its own attention, pair1 adds them back
"""
import math
import ml_dtypes
import numpy as np

import concourse.bass as bass
from concourse import bacc
import concourse.mybir as mybir
import concourse.tile as tile
from concourse.bass_utils import run_bass_kernel_spmd
from concourse.masks import make_identity

F32 = mybir.dt.float32
F32R = mybir.dt.float32r
F16 = mybir.dt.float16
BF16 = mybir.dt.bfloat16
AF = mybir.ActivationFunctionType

B, L, C, H, D = 2, 2048, 1024, 16, 64
NCORES = 8
ROPE_THETA = 10000.0
RMS_EPS = 1e-6
NPAIR = 2        # head pairs per core
LQB = 512        # q block size
NQB = L // LQB   # 4 q blocks
NKV = L // 128   # 16 kv chunks


def _build_program():
    nc = bacc.Bacc("TRN2", target_bir_lowering=False, debug=False)

    xt_d = nc.dram_tensor("xt", [C, L], BF16, kind="ExternalInput")
    wq_d = nc.dram_tensor("wq", [C, 768], BF16, kind="ExternalInput")
    bias_d = nc.dram_tensor("bias6", [6, 128], F32, kind="ExternalInput")
    cos_d = nc.dram_tensor("cost", [128, L], BF16, kind="ExternalInput")
    sin_d = nc.dram_tensor("sint", [128, L], BF16, kind="ExternalInput")
    ind_d = nc.dram_tensor("ind", [128, 4], BF16, kind="ExternalInput")
    gind_d = nc.dram_tensor("gind", [4, 4 * 128], F32R, kind="ExternalInput")
    ones_d = nc.dram_tensor("ones128", [128, 64], F32R, kind="ExternalInput")
    wo_d = nc.dram_tensor("wo", [256, 1024], BF16, kind="ExternalInput")
    out_d = nc.dram_tensor("out", [L, C], F16, kind="ExternalOutput")

    with tile.TileContext(nc) as tc:
        with tc.tile_pool(name="const", bufs=1) as cp:
            # ---- constant tiles (DMAs for late-use consts issued after the
            # hot-path wq/xt loads so the first QKV chains start ASAP) ----
            cos_t = cp.tile([128, L], BF16, tag="cos")
            sin_t = cp.tile([128, L], BF16, tag="sin")
            ind_t = cp.tile([128, 4], BF16, tag="ind")
            gind_t = cp.tile([4, 4 * 128], F32R, tag="gind")
            ones_t = cp.tile([128, 64], F32R, tag="ones")
            wo_t = [cp.tile([128, 1024], BF16, tag=f"wo{p}", name=f"wo{p}")
                    for p in range(2)]
            bias_t = cp.tile([128, 6], F32, tag="bias")
            lnb_t = cp.tile([4, 1], F32, tag="lnb")
            nc.vector.memset(lnb_t[:, :], 0.0)
            nc.vector.memset(lnb_t[0:2, :], -math.log(8.0))
            eps_t = cp.tile([4, 1], F32, tag="eps")
            nc.vector.memset(eps_t[:, :], RMS_EPS)
            ident = cp.tile([128, 128], BF16, tag="ident")
            make_identity(nc, ident[:, :])
            onecol = cp.tile([128, 2], F32, tag="onecol")
            nc.vector.memset(onecol[:, :], 1.0)

            # xw below the chunk pools on the right stack: released last
            # (from inside the attention filler, after chkD)
            xw = tc.alloc_tile_pool(name="xw", bufs=1, side="right")
            # E/O/V chunks per pair: rows of E = [qA_e, qB_e, kA_e, kB_e]
            chkD = tc.alloc_tile_pool(name="chkD", bufs=1, side="right")
            chk = tc.alloc_tile_pool(name="chk", bufs=1, side="right")
            chunks = [
                (chkD if i in (3, 4, 5) else chk).tile(
                    [128, L], BF16,
                    tag=f"c{i}", name=f"c{i}") for i in range(6)]

            # ---- phase 1: fused QKV projection (pair0 chunks up front;
            # pair1's chains run as attention filler, xw stays alive) ----
            with tc.tile_pool(name="psq", bufs=3, space="PSUM") as psq:
                wq_sb = []
                for cc in range(8):
                    wqi = xw.tile([128, 768], BF16, tag=f"w{cc}", name=f"w{cc}")
                    nc.sync.dma_start(out=wqi, in_=wq_d[cc * 128:(cc + 1) * 128, :])
                    wq_sb.append(wqi)
                nc.sync.dma_start(out=bias_t, in_=bias_d[:, :].transpose([1, 0]))
                xt_sb = [[None] * 4 for _ in range(8)]
                for lq in range(4):
                    for cc in range(8):
                        xti = xw.tile([128, 512], BF16, tag=f"x{cc}_{lq}",
                                      name=f"x{cc}_{lq}")
                        nc.sync.dma_start(
                            out=xti,
                            in_=xt_d[cc * 128:(cc + 1) * 128,
                                     lq * 512:(lq + 1) * 512])
                        xt_sb[cc][lq] = xti
                # late-use constants after the hot path
                nc.sync.dma_start(out=ind_t, in_=ind_d[:, :])
                nc.sync.dma_start(out=cos_t, in_=cos_d[:, :])
                nc.sync.dma_start(out=sin_t, in_=sin_d[:, :])
                nc.sync.dma_start(out=gind_t, in_=gind_d[:, :])
                nc.sync.dma_start(out=ones_t, in_=ones_d[:, :])
                for p in range(2):
                    nc.sync.dma_start(out=wo_t[p],
                                      in_=wo_d[p * 128:(p + 1) * 128, :])
                for lq in range(4):
                    for oc in range(3):
                        ps = psq.tile([128, 512], F32, tag="ps")
                        for cc in range(8):
                            nc.tensor.matmul(
                                ps[:, :],
                                wq_sb[cc][:, oc * 128:(oc + 1) * 128],
                                xt_sb[cc][lq][:, :],
                                start=(cc == 0), stop=(cc == 7),
                            )
                        nc.vector.tensor_scalar_add(
                            chunks[oc][:, lq * 512:(lq + 1) * 512],
                            ps[:, :], bias_t[:, oc:oc + 1])

            # long-lived attention operands (allocated after xw released)
            lv = tc.alloc_tile_pool(name="live", bufs=1)
            qT, kT, vseq = [], [], []
            for p in range(NPAIR):
                qT.append(lv.tile([128, L], BF16, tag=f"qT{p}", name=f"qT{p}"))
                kT.append(lv.tile([128, L], BF16, tag=f"kT{p}", name=f"kT{p}"))
                vseq.append([lv.tile([128, 130], BF16, tag=f"vs{p}_{lw}",
                                     name=f"vs{p}_{lw}") for lw in range(NKV)])
            # normalized attention outputs, per (pair, q-block)
            oq = [[lv.tile([128, LQB], BF16, tag=f"oq{p}_{qb}",
                           name=f"oq{p}_{qb}") for qb in range(NQB)]
                  for p in range(NPAIR)]

            # ---- phase 2: pair0 rope + rmsnorm + relocation + v transpose
            # (pair1's prep is deferred into pair0's attention as filler) ----
            dfr = tc.alloc_tile_pool(name="dfr", bufs=1)
            M_sb = [dfr.tile([128, L], BF16, tag=f"Msb{k}", name=f"Msb{k}")
                    for k in range(2)]
            with (
                tc.tile_pool(name="tmp", bufs=1) as tp,
                tc.tile_pool(name="psp", bufs=2, space="PSUM") as psp,
            ):
                E, O, V = chunks[0], chunks[1], chunks[2]
                sqE = tp.tile([128, L], BF16, tag="bigA")
                nc.vector.tensor_mul(sqE[:, :], E[:, :], E[:, :])
                sqO = tp.tile([128, L], BF16, tag="bigB")
                nc.vector.tensor_mul(sqO[:, :], O[:, :], O[:, :])
                # batch all Ln calls, then all Exp calls: Ln and Exp live in
                # different ACT table sets, and every alternation costs a
                # ~1.3us ACT_TABLE_LOAD
                lnvs = []
                for lw in range(4):
                    ps4 = psp.tile([4, 512], F32, tag="ps4")
                    nc.tensor.matmul(ps4[:, :], ind_t[:, :],
                                     sqE[:, lw * 512:(lw + 1) * 512],
                                     start=True, stop=False)
                    nc.tensor.matmul(ps4[:, :], ind_t[:, :],
                                     sqO[:, lw * 512:(lw + 1) * 512],
                                     start=False, stop=True)
                    lnv = tp.tile([4, 512], F32, tag=f"lnv{lw}",
                                  name=f"lnv{lw}")
                    nc.scalar.activation(lnv[:, :], ps4[:, :], AF.Ln,
                                         scale=1.0 / 64.0, bias=eps_t[:, 0:1])
                    lnvs.append(lnv)
                invrs = []
                for lw in range(4):
                    iv = tp.tile([4, 512], F32R, tag=f"inv{lw}",
                                 name=f"inv{lw}")
                    nc.scalar.activation(iv[:, :], lnvs[lw][:, :], AF.Exp,
                                         scale=-0.5, bias=lnb_t[:, 0:1])
                    invrs.append(iv)
                # rope now (bf16, 3 temps: A, B, C)
                t1c = tp.tile([128, L], BF16, tag="bigC")
                nc.vector.tensor_mul(t1c[:, :], E[:, :], cos_t[:, :])
                t2s = tp.tile([128, L], BF16, tag="bigB")
                nc.vector.tensor_mul(t2s[:, :], O[:, :], sin_t[:, :])
                rE = tp.tile([128, L], BF16, tag="bigA")
                nc.vector.tensor_sub(rE[:, :], t1c[:, :], t2s[:, :])
                t1s = tp.tile([128, L], BF16, tag="bigC")
                nc.vector.tensor_mul(t1s[:, :], E[:, :], sin_t[:, :])
                t2c = tp.tile([128, L], BF16, tag="bigB")
                nc.vector.tensor_mul(t2c[:, :], O[:, :], cos_t[:, :])
                rO = t1s
                nc.vector.tensor_add(rO[:, :], t1s[:, :], t2c[:, :])
                sE = tp.tile([128, L], BF16, tag="sc16E")
                sO = tp.tile([128, L], BF16, tag="sc16O")
                for kind, (rt, st) in enumerate(((rE, sE), (rO, sO))):
                    gsl = gind_t[:, kind * 128:(kind + 1) * 128]
                    for lw in range(4):
                        mm = psp.tile([128, 512], F32, tag="mps")
                        nc.tensor.matmul(mm[:, :], gsl,
                                         invrs[lw][:, :],
                                         start=True, stop=True)
                        nc.vector.tensor_mul(
                            st[:, lw * 512:(lw + 1) * 512],
                            rt[:, lw * 512:(lw + 1) * 512], mm[:, :])
                for blk in range(2):
                    nc.sync.dma_start(out=qT[0][blk * 64:blk * 64 + 32, :],
                                      in_=sE[blk * 32:(blk + 1) * 32, :])
                    nc.sync.dma_start(out=qT[0][blk * 64 + 32:blk * 64 + 64, :],
                                      in_=sO[blk * 32:(blk + 1) * 32, :])
                    nc.sync.dma_start(out=kT[0][blk * 64:blk * 64 + 32, :],
                                      in_=sE[64 + blk * 32:64 + (blk + 1) * 32, :])
                    nc.sync.dma_start(out=kT[0][blk * 64 + 32:blk * 64 + 64, :],
                                      in_=sO[64 + blk * 32:64 + (blk + 1) * 32, :])
                # v transpose -> vseq [l,130]: [vA(64) 1 vB(64) 1]
                for lw in range(NKV):
                    pt = psp.tile([128, 128], BF16, tag="ptr")
                    nc.tensor.transpose(pt[:, :],
                                        V[:, lw * 128:(lw + 1) * 128],
                                        ident[:, :])
                    vv = vseq[0][lw].rearrange("a (h x) -> a h x", h=2)
                    nc.vector.tensor_copy(
                        vv[:, :, 0:64],
                        pt[:, :].rearrange("a (h x) -> a h x", h=2))
                    nc.vector.tensor_copy(vv[:, :, 64], onecol[:, :])

            chk.release()
            # pair0 half of the output projection, bf16 partials
            pav = tc.alloc_tile_pool(name="pav", bufs=1)
            pA = [[pav.tile([128, 512], BF16, tag=f"pA{qb}_{i}",
                            name=f"pA{qb}_{i}") for i in range(8)]
                  for qb in range(NQB)]
            dfr2 = tc.alloc_tile_pool(name="dfr2", bufs=1)
            rA = dfr2.tile([128, L], BF16, tag="rA")
            rB = dfr2.tile([128, L], BF16, tag="rB")
            sE1 = dfr2.tile([128, L], BF16, tag="sE1")
            sO1 = dfr2.tile([128, L], BF16, tag="sO1")
            lnv1 = [dfr2.tile([4, 512], F32, tag=f"lnv1_{lw}",
                              name=f"lnv1_{lw}") for lw in range(4)]
            iv1 = [dfr2.tile([4, 512], F32R, tag=f"iv1_{lw}",
                             name=f"iv1_{lw}") for lw in range(4)]
            E1, O1, V1 = chunks[3], chunks[4], chunks[5]

            # pair1 prep, dripped one item per attention window of pair0.
            # PSUM scratch comes from the attention-phase "po" slots, so the
            # closures must run inside the attention pool scope.
            # Filler ordering matters: engines execute in program order, so a
            # filler op whose producer (on another engine) hasn't run yet
            # head-blocks every later op on its engine — including the EXP
            # stream. Squares go first; ACT-free transpose items pad the gap
            # until the squares have certainly retired; only then the
            # Ln/Exp stats; then the gamma broadcasts; then rope.
            def _mk_deferred(pso_tile):
                u = []

                def qkv1(oc, lq):
                    ps = pso_tile([128, 512], F32)
                    for cc in range(8):
                        nc.tensor.matmul(
                            ps[:, :],
                            wq_sb[cc][:, oc * 128:(oc + 1) * 128],
                            xt_sb[cc][lq][:, :],
                            start=(cc == 0), stop=(cc == 7),
                        )
                    nc.vector.tensor_scalar_add(
                        chunks[oc][:, lq * 512:(lq + 1) * 512],
                        ps[:, :], bias_t[:, oc:oc + 1])
                for oc in (3, 4, 5):
                    for lq in range(4):
                        u.append((1700, lambda oc=oc, lq=lq: qkv1(oc, lq)))

                u.append((0, lambda: nc.vector.tensor_mul(rA[:, :], E1[:, :], E1[:, :])))
                u.append((0, lambda: nc.vector.tensor_mul(rB[:, :], O1[:, :], O1[:, :])))

                def stats_ln():
                    # one item: all four Ln calls back-to-back (1 table load)
                    for lw in range(4):
                        ps4 = pso_tile([4, 512], F32)
                        nc.tensor.matmul(ps4[:, :], ind_t[:, :],
                                         rA[:, lw * 512:(lw + 1) * 512],
                                         start=True, stop=False)
                        nc.tensor.matmul(ps4[:, :], ind_t[:, :],
                                         rB[:, lw * 512:(lw + 1) * 512],
                                         start=False, stop=True)
                        nc.scalar.activation(lnv1[lw][:, :], ps4[:, :], AF.Ln,
                                             scale=1.0 / 64.0,
                                             bias=eps_t[:, 0:1])

                def stats_exp():
                    for lw in range(4):
                        nc.scalar.activation(iv1[lw][:, :], lnv1[lw][:, :],
                                             AF.Exp, scale=-0.5,
                                             bias=lnb_t[:, 0:1])

                def msb(kind, lw):
                    gsl = gind_t[:, (2 + kind) * 128:(2 + kind + 1) * 128]
                    mm = pso_tile([128, 512], F32)
                    nc.tensor.matmul(mm[:, :], gsl, iv1[lw][:, :],
                                     start=True, stop=True)
                    nc.vector.tensor_copy(
                        M_sb[kind][:, lw * 512:(lw + 1) * 512], mm[:, :])

                def vtr(lw):
                    pt = pso_tile([128, 128], BF16)
                    nc.tensor.transpose(pt[:, :], V1[:, lw * 128:(lw + 1) * 128],
                                        ident[:, :])
                    vv = vseq[1][lw].rearrange("a (h x) -> a h x", h=2)
                    nc.vector.tensor_copy(
                        vv[:, :, 0:64],
                        pt[:, :].rearrange("a (h x) -> a h x", h=2))
                    nc.vector.tensor_copy(vv[:, :, 64], onecol[:, :])

                for lw in range(10):
                    u.append((100, lambda lw=lw: vtr(lw)))
                u.append((900, stats_ln))
                for lw in range(10, 13):
                    u.append((100, lambda lw=lw: vtr(lw)))
                u.append((0, stats_exp))
                for lw in range(13, NKV):
                    u.append((100, lambda lw=lw: vtr(lw)))
                for kind in range(2):
                    for lw in range(4):
                        u.append((250, lambda k=kind, lw=lw: msb(k, lw)))

                u.append((0, lambda: nc.vector.tensor_mul(rA[:, :], E1[:, :], cos_t[:, :])))
                u.append((0, lambda: nc.vector.tensor_mul(rB[:, :], O1[:, :], sin_t[:, :])))
                u.append((0, lambda: nc.vector.tensor_sub(rA[:, :], rA[:, :], rB[:, :])))
                u.append((0, lambda: nc.vector.tensor_mul(sE1[:, :], rA[:, :], M_sb[0][:, :])))
                u.append((0, lambda: nc.vector.tensor_mul(rA[:, :], E1[:, :], sin_t[:, :])))
                u.append((0, lambda: nc.vector.tensor_mul(rB[:, :], O1[:, :], cos_t[:, :])))
                u.append((0, lambda: nc.vector.tensor_add(rA[:, :], rA[:, :], rB[:, :])))
                u.append((0, lambda: nc.vector.tensor_mul(sO1[:, :], rA[:, :], M_sb[1][:, :])))
                for blk in range(2):
                    for args in (
                        (qT[1][blk * 64:blk * 64 + 32, :], sE1[blk * 32:(blk + 1) * 32, :]),
                        (qT[1][blk * 64 + 32:blk * 64 + 64, :], sO1[blk * 32:(blk + 1) * 32, :]),
                        (kT[1][blk * 64:blk * 64 + 32, :], sE1[64 + blk * 32:64 + (blk + 1) * 32, :]),
                        (kT[1][blk * 64 + 32:blk * 64 + 64, :], sO1[64 + blk * 32:64 + (blk + 1) * 32, :]),
                    ):
                        u.append((0, lambda a=args: nc.sync.dma_start(out=a[0], in_=a[1])))
                u.append((0, lambda: chkD.release()))
                u.append((0, lambda: xw.release()))
                return u

            # ---- phase 3: attention + fused output projection ----
            # Per (pair, q-block of 512): flash loop over 16 kv chunks.
            # Emission is software-pipelined (scores[j+1] ahead of PV[j]) so
            # the PE stream never head-blocks on the EXP; the out-projection
            # for q-block qb is dripped into the following windows as PE
            # filler (keeps the HAM clock warm).
            with (
                tc.tile_pool(name="exp", bufs=6) as xp,
                tc.tile_pool(name="den", bufs=3) as dn,
                tc.tile_pool(name="oev", bufs=3) as oe,
                tc.tile_pool(name="psa", bufs=2, space="PSUM") as psa,
                tc.tile_pool(name="psv", bufs=1, space="PSUM") as psv,
                tc.tile_pool(name="pso", bufs=2, space="PSUM") as pso,
            ):
                _poctr = [0]

                def _po_tile(shape, dt):
                    _poctr[0] += 1
                    return pso.tile(shape, dt, tag="po",
                                    name=f"pof{_poctr[0]}")

                filler = _mk_deferred(_po_tile)

                # out-projection split across pairs: pair0's half runs as
                # filler during its own attention (PSUM -> bf16 SBUF
                # partials), pair1's half adds them back. Both phases get a
                # real matmul per window, which keeps the PE HAM clock warm.
                def mk_proj0(qb):
                    def one(i, lw, n):
                        def run():
                            po = _po_tile([128, 512], F32)
                            nc.tensor.matmul(
                                po[:, :],
                                oq[0][qb][:, lw * 128:(lw + 1) * 128],
                                wo_t[0][:, n * 512:(n + 1) * 512],
                                start=True, stop=True)
                            nc.vector.tensor_copy(pA[qb][i][:, :], po[:, :])
                        return run
                    return [(450, one(i, lw, n))
                            for i, (lw, n) in enumerate(
                                (lw, n) for lw in range(4) for n in range(2))]

                def mk_proj1(qb):
                    def one(i, lw, n):
                        def run():
                            po = _po_tile([128, 512], F32)
                            nc.tensor.matmul(
                                po[:, :],
                                oq[1][qb][:, lw * 128:(lw + 1) * 128],
                                wo_t[1][:, n * 512:(n + 1) * 512],
                                start=True, stop=True)
                            ov = oe.tile([128, 512], F16, tag="ov")
                            nc.vector.tensor_add(ov[:, :], po[:, :],
                                                 pA[qb][i][:, :])
                            nc.sync.dma_start(
                                out=out_d[qb * LQB + lw * 128:
                                          qb * LQB + (lw + 1) * 128,
                                          n * 512:(n + 1) * 512],
                                in_=ov[:, :])
                        return run
                    return [(450, one(i, lw, n))
                            for i, (lw, n) in enumerate(
                                (lw, n) for lw in range(4) for n in range(2))]

                for p in range(NPAIR):
                    if p == 1:
                        while filler:
                            filler.pop(0)[1]()
                    for qb in range(NQB):
                        q0 = qb * LQB
                        oA = psv.tile([65, LQB], F32, tag="oA")
                        oB = psv.tile([65, LQB], F32, tag="oB")

                        def emit_scores(j):
                            sAB = psa.tile([128, 2 * LQB], F32, tag="sAB")
                            nc.tensor.matmul(
                                sAB[:, 0:LQB],
                                kT[p][0:64, j * 128:(j + 1) * 128],
                                qT[p][0:64, q0:q0 + LQB],
                                start=True, stop=True, tile_position=(0, 0))
                            nc.tensor.matmul(
                                sAB[:, LQB:2 * LQB],
                                kT[p][64:128, j * 128:(j + 1) * 128],
                                qT[p][64:128, q0:q0 + LQB],
                                start=True, stop=True, tile_position=(64, 0))
                            return sAB

                        def emit_pv(j, eAB):
                            nc.tensor.matmul(
                                oA[:, :], vseq[p][j][:, 0:65],
                                eAB[:, 0:LQB],
                                start=(j == 0), stop=(j == NKV - 1))
                            nc.tensor.matmul(
                                oB[:, :], vseq[p][j][:, 65:130],
                                eAB[:, LQB:2 * LQB],
                                start=(j == 0), stop=(j == NKV - 1))

                        prev_e = None
                        for j in range(NKV):
                            sAB = emit_scores(j)
                            if prev_e is not None:
                                emit_pv(j - 1, prev_e)
                            eAB = xp.tile([128, 2 * LQB], BF16, tag="eAB")
                            nc.scalar.activation(eAB[:, :], sAB[:, :], AF.Exp)
                            prev_e = eAB
                            got = 0
                            if filler:
                                w, fn = filler.pop(0)
                                fn()
                                got += w
                            if len(filler) > 20:
                                w, fn = filler.pop(0)
                                fn()
                                got += w
                        emit_pv(NKV - 1, prev_e)

                        # normalize: o / denom (denom = row 64); the
                        # ones-row matmul broadcast beats a DMA+GpSimd route
                        # (the DMA round-trip stalls each q-block boundary)
                        for hd, oo in enumerate((oA, oB)):
                            den = dn.tile([65, LQB], F32R, tag="den")
                            nc.vector.tensor_copy(den[64:65, :], oo[64:65, :])
                            dbc = pso.tile([64, LQB], F32, tag="po")
                            nc.tensor.matmul(
                                dbc[:, :], ones_t[64:65, :],
                                den[64:65, :], start=True, stop=True)
                            rcb = dn.tile([64, LQB], F32, tag="rcb")
                            nc.vector.reciprocal_approx_fast(rcb[:, :], dbc[:, :])
                            onrm = dn.tile([64, LQB], BF16, tag="onrm")
                            nc.vector.tensor_mul(onrm[:, :], oo[0:64, :], rcb[:, :])
                            nc.sync.dma_start(
                                out=oq[p][qb][hd * 64:(hd + 1) * 64, :],
                                in_=onrm[:, :])
                        filler.extend(
                            (mk_proj0 if p == 0 else mk_proj1)(qb))
                while filler:
                    filler.pop(0)[1]()


            dfr2.release()
            pav.release()
            dfr.release()
            lv.release()
    nc.compile()
    return nc


_PROG = None


def _get_program():
    global _PROG
    if _PROG is None:
        _PROG = _build_program()
    return _PROG


def _core_inputs(core, x, w_qkv, b_qkv, w_out, q_gamma, k_gamma,
                 cos_tab, sin_tab, ind, ones128):
    b = core // 4
    hb = (core % 4) * 4
    # row permutation of w_qkv for this core
    perm = []
    for p in range(NPAIR):
        hA, hB = hb + 2 * p, hb + 2 * p + 1
        for h in (hA, hB):                      # E chunk: q evens, k evens
            perm += [h * 64 + 2 * j for j in range(32)]
        for h in (hA, hB):
            perm += [1024 + h * 64 + 2 * j for j in range(32)]
        for h in (hA, hB):                      # O chunk
            perm += [h * 64 + 2 * j + 1 for j in range(32)]
        for h in (hA, hB):
            perm += [1024 + h * 64 + 2 * j + 1 for j in range(32)]
        for h in (hA, hB):                      # V chunk
            perm += [2048 + h * 64 + c for c in range(64)]
    perm = np.asarray(perm)
    w_local = w_qkv[perm]                       # [768, 1024]
    wq = np.ascontiguousarray(w_local.T)        # [1024, 768]
    bias6 = np.ascontiguousarray(b_qkv[perm].reshape(6, 128))

    # gamma-scaled indicator lhsT [4, 4*128]: (p, kind) -> [4, 128]
    gind = np.zeros((4, 4 * 128), np.float32)
    for p in range(NPAIR):
        for kind in range(2):                   # 0=E(evens), 1=O(odds)
            blk = (2 * p + kind) * 128
            for r in range(128):
                g = r // 32
                h = hb + 2 * p + (g % 2)
                ch = 2 * (r % 32) + kind
                gam = q_gamma[h, ch] if g < 2 else k_gamma[h, ch]
                gind[g, blk + r] = gam

    # w_out slice: [256, 1024]
    wo = np.empty((256, 1024), np.float32)
    for p in range(NPAIR):
        for i, h in enumerate((hb + 2 * p, hb + 2 * p + 1)):
            wo[p * 128 + i * 64:p * 128 + (i + 1) * 64, :] = \
                w_out[:, h * 64:(h + 1) * 64].T

    return {
        "xt": np.ascontiguousarray(x[b].T).astype(ml_dtypes.bfloat16),
        "wq": wq.astype(ml_dtypes.bfloat16),
        "bias6": bias6,
        "cost": cos_tab,
        "sint": sin_tab,
        "ind": ind.astype(ml_dtypes.bfloat16),
        "gind": gind,
        "ones128": ones128,
        "wo": wo.astype(ml_dtypes.bfloat16),
    }


def kernel(x, w_qkv, b_qkv, w_out, b_out, q_gamma, k_gamma, _trace=False):
    x = np.asarray(x, np.float32)
    w_qkv = np.asarray(w_qkv, np.float32)
    b_qkv = np.asarray(b_qkv, np.float32)
    w_out = np.asarray(w_out, np.float32)
    b_out = np.asarray(b_out, np.float32)
    q_gamma = np.asarray(q_gamma, np.float32)
    k_gamma = np.asarray(k_gamma, np.float32)

    inv_freq = (1.0 / ROPE_THETA ** (np.arange(32, dtype=np.float64) / 32.0))
    ang = np.arange(L, dtype=np.float64)[None, :] * \
        np.tile(inv_freq, 4)[:, None]          # [128, L], row r -> freq r%32
    cos_tab = np.cos(ang).astype(ml_dtypes.bfloat16)
    sin_tab = np.sin(ang).astype(ml_dtypes.bfloat16)
    ind = np.zeros((128, 4), np.float32)
    for r in range(128):
        ind[r, r // 32] = 1.0
    ones128 = np.ones((128, 64), np.float32)

    nc = _get_program()
    in_maps = [_core_inputs(c, x, w_qkv, b_qkv, w_out, q_gamma, k_gamma,
                            cos_tab, sin_tab, ind, ones128)
               for c in range(NCORES)]
    r = run_bass_kernel_spmd(nc, in_maps, list(range(NCORES)), trace=_trace)
    out = np.zeros((B, L, C), np.float32)
    for c in range(NCORES):
        out[c // 4] += r.results[c]["out"].astype(np.float32)
    out += b_out[None, None, :]
    if _trace:
        kernel._last_results = r
    return out



# revision 54
# speedup vs baseline: 1.1206x; 1.0105x over previous
"""MultiHeadAttention (RoPE + QK-RMSNorm, non-causal) on 8 trn2 NeuronCores.

Sharding: batch (2) x head-groups (4 heads each) -> 8 cores. Each core:
  - QKV projection for its 4 heads (768 output channels) from x[b] (full seq)
  - RoPE + QK-RMSNorm fused via channel permutation + per-row scale matmuls
  - full 2048x2048 attention for 4 heads (flash-style, scores transposed,
    softmax without max-subtraction: RMS-normed logits are bounded by 8)
  - output projection partial (its 256 channels of the 1024-ch contraction)
Host: sums the 4 partials per batch (fp16 device outputs) and adds b_out.

Performance structure (tuned against perfetto + HAM traces):
  - every matmul operand is bf16 (fp16 streams at HALF the PE rate on TRN2;
    bf16/f32r run the full 2.4 GHz column rate)
  - the attention j-loop is ScalarE-bound (EXP at (N+352)/1.2 ns); emission
    is software-pipelined (scores[j+1] before PV[j]) and everything movable
    (pair1 QKV chains, its RMS stats / gamma broadcasts / V transposes /
    rope, and the split output projection) drips into the windows as PE
    filler so the PE HAM clock stays at 8/8
  - Ln and Exp live in different ACT table sets (~1.3us reload per switch),
    so the RMS-norm batches all Lns, then all Exps
  - the output projection is split per pair: pair0's half accumulates to
    bf16 SBUF partials during its own attention, pair1 adds them back
"""
import math
import ml_dtypes
import numpy as np

import concourse.bass as bass
from concourse import bacc
import concourse.mybir as mybir
import concourse.tile as tile
from concourse.bass_utils import run_bass_kernel_spmd
from concourse.masks import make_identity

F32 = mybir.dt.float32
F32R = mybir.dt.float32r
F16 = mybir.dt.float16
BF16 = mybir.dt.bfloat16
AF = mybir.ActivationFunctionType

B, L, C, H, D = 2, 2048, 1024, 16, 64
NCORES = 8
ROPE_THETA = 10000.0
RMS_EPS = 1e-6
NPAIR = 2        # head pairs per core
LQB = 512        # q block size
NQB = L // LQB   # 4 q blocks
NKV = L // 128   # 16 kv chunks


def _build_program():
    nc = bacc.Bacc("TRN2", target_bir_lowering=False, debug=False)

    xt_d = nc.dram_tensor("xt", [C, L], BF16, kind="ExternalInput")
    wq_d = nc.dram_tensor("wq", [C, 768], BF16, kind="ExternalInput")
    bias_d = nc.dram_tensor("bias6", [6, 128], F32, kind="ExternalInput")
    cos_d = nc.dram_tensor("cost", [128, L], BF16, kind="ExternalInput")
    sin_d = nc.dram_tensor("sint", [128, L], BF16, kind="ExternalInput")
    ind_d = nc.dram_tensor("ind", [128, 4], BF16, kind="ExternalInput")
    gind_d = nc.dram_tensor("gind", [4, 4 * 128], F32R, kind="ExternalInput")
    ones_d = nc.dram_tensor("ones128", [128, 64], F32R, kind="ExternalInput")
    wo_d = nc.dram_tensor("wo", [256, 1024], BF16, kind="ExternalInput")
    out_d = nc.dram_tensor("out", [L, C], F16, kind="ExternalOutput")

    with tile.TileContext(nc) as tc:
        with tc.tile_pool(name="const", bufs=1) as cp:
            # ---- constant tiles (DMAs for late-use consts issued after the
            # hot-path wq/xt loads so the first QKV chains start ASAP) ----
            cos_t = cp.tile([128, L], BF16, tag="cos")
            sin_t = cp.tile([128, L], BF16, tag="sin")
            ind_t = cp.tile([128, 4], BF16, tag="ind")
            gind_t = cp.tile([4, 4 * 128], F32R, tag="gind")
            ones_t = cp.tile([128, 64], F32R, tag="ones")
            wo_t = [cp.tile([128, 1024], BF16, tag=f"wo{p}", name=f"wo{p}")
                    for p in range(2)]
            bias_t = cp.tile([128, 6], F32, tag="bias")
            lnb_t = cp.tile([4, 1], F32, tag="lnb")
            nc.vector.memset(lnb_t[:, :], 0.0)
            nc.vector.memset(lnb_t[0:2, :], -math.log(8.0))
            eps_t = cp.tile([4, 1], F32, tag="eps")
            nc.vector.memset(eps_t[:, :], RMS_EPS)
            ident = cp.tile([128, 128], BF16, tag="ident")
            make_identity(nc, ident[:, :])
            onecol = cp.tile([128, 2], F32, tag="onecol")
            nc.vector.memset(onecol[:, :], 1.0)

            # xw below the chunk pools on the right stack: released last
            # (from inside the attention filler, after chkD)
            xw = tc.alloc_tile_pool(name="xw", bufs=1, side="right")
            # E/O/V chunks per pair: rows of E = [qA_e, qB_e, kA_e, kB_e]
            chkD = tc.alloc_tile_pool(name="chkD", bufs=1, side="right")
            chk = tc.alloc_tile_pool(name="chk", bufs=1, side="right")
            chunks = [
                (chkD if i in (3, 4, 5) else chk).tile(
                    [128, L], BF16,
                    tag=f"c{i}", name=f"c{i}") for i in range(6)]

            # ---- phase 1: fused QKV projection (pair0 chunks up front;
            # pair1's chains run as attention filler, xw stays alive) ----
            with tc.tile_pool(name="psq", bufs=3, space="PSUM") as psq:
                wq_sb = []
                for cc in range(8):
                    wqi = xw.tile([128, 768], BF16, tag=f"w{cc}", name=f"w{cc}")
                    nc.sync.dma_start(out=wqi, in_=wq_d[cc * 128:(cc + 1) * 128, :])
                    wq_sb.append(wqi)
                nc.sync.dma_start(out=bias_t, in_=bias_d[:, :].transpose([1, 0]))
                xt_sb = [[None] * 4 for _ in range(8)]
                for lq in range(4):
                    for cc in range(8):
                        xti = xw.tile([128, 512], BF16, tag=f"x{cc}_{lq}",
                                      name=f"x{cc}_{lq}")
                        nc.sync.dma_start(
                            out=xti,
                            in_=xt_d[cc * 128:(cc + 1) * 128,
                                     lq * 512:(lq + 1) * 512])
                        xt_sb[cc][lq] = xti
                # late-use constants after the hot path
                nc.sync.dma_start(out=ind_t, in_=ind_d[:, :])
                nc.sync.dma_start(out=cos_t, in_=cos_d[:, :])
                nc.sync.dma_start(out=sin_t, in_=sin_d[:, :])
                nc.sync.dma_start(out=gind_t, in_=gind_d[:, :])
                nc.sync.dma_start(out=ones_t, in_=ones_d[:, :])
                for p in range(2):
                    nc.sync.dma_start(out=wo_t[p],
                                      in_=wo_d[p * 128:(p + 1) * 128, :])
                for lq in range(4):
                    for oc in range(3):
                        ps = psq.tile([128, 512], F32, tag="ps")
                        for cc in range(8):
                            nc.tensor.matmul(
                                ps[:, :],
                                wq_sb[cc][:, oc * 128:(oc + 1) * 128],
                                xt_sb[cc][lq][:, :],
                                start=(cc == 0), stop=(cc == 7),
                            )
                        nc.vector.tensor_scalar_add(
                            chunks[oc][:, lq * 512:(lq + 1) * 512],
                            ps[:, :], bias_t[:, oc:oc + 1])

            # long-lived attention operands (allocated after xw released)
            lv = tc.alloc_tile_pool(name="live", bufs=1)
            qT, kT, vseq = [], [], []
            for p in range(NPAIR):
                qT.append(lv.tile([128, L], BF16, tag=f"qT{p}", name=f"qT{p}"))
                kT.append(lv.tile([128, L], BF16, tag=f"kT{p}", name=f"kT{p}"))
                vseq.append([lv.tile([128, 130], BF16, tag=f"vs{p}_{lw}",
                                     name=f"vs{p}_{lw}") for lw in range(NKV)])
            # normalized attention outputs, per (pair, q-block)
            oq = [[lv.tile([128, LQB], BF16, tag=f"oq{p}_{qb}",
                           name=f"oq{p}_{qb}") for qb in range(NQB)]
                  for p in range(NPAIR)]

            # ---- phase 2: pair0 rope + rmsnorm + relocation + v transpose
            # (pair1's prep is deferred into pair0's attention as filler) ----
            dfr = tc.alloc_tile_pool(name="dfr", bufs=1)
            M_sb = [dfr.tile([128, L], BF16, tag=f"Msb{k}", name=f"Msb{k}")
                    for k in range(2)]
            with (
                tc.tile_pool(name="tmp", bufs=1) as tp,
                tc.tile_pool(name="psp", bufs=2, space="PSUM") as psp,
            ):
                E, O, V = chunks[0], chunks[1], chunks[2]
                sqE = tp.tile([128, L], BF16, tag="bigA")
                nc.vector.tensor_mul(sqE[:, :], E[:, :], E[:, :])
                sqO = tp.tile([128, L], BF16, tag="bigB")
                nc.vector.tensor_mul(sqO[:, :], O[:, :], O[:, :])
                # batch all Ln calls, then all Exp calls: Ln and Exp live in
                # different ACT table sets, and every alternation costs a
                # ~1.3us ACT_TABLE_LOAD
                lnvs = []
                for lw in range(4):
                    ps4 = psp.tile([4, 512], F32, tag="ps4")
                    nc.tensor.matmul(ps4[:, :], ind_t[:, :],
                                     sqE[:, lw * 512:(lw + 1) * 512],
                                     start=True, stop=False)
                    nc.tensor.matmul(ps4[:, :], ind_t[:, :],
                                     sqO[:, lw * 512:(lw + 1) * 512],
                                     start=False, stop=True)
                    lnv = tp.tile([4, 512], F32, tag=f"lnv{lw}",
                                  name=f"lnv{lw}")
                    nc.scalar.activation(lnv[:, :], ps4[:, :], AF.Ln,
                                         scale=1.0 / 64.0, bias=eps_t[:, 0:1])
                    lnvs.append(lnv)
                invrs = []
                for lw in range(4):
                    iv = tp.tile([4, 512], F32R, tag=f"inv{lw}",
                                 name=f"inv{lw}")
                    nc.scalar.activation(iv[:, :], lnvs[lw][:, :], AF.Exp,
                                         scale=-0.5, bias=lnb_t[:, 0:1])
                    invrs.append(iv)
                # rope now (bf16, 3 temps: A, B, C)
                t1c = tp.tile([128, L], BF16, tag="bigC")
                nc.vector.tensor_mul(t1c[:, :], E[:, :], cos_t[:, :])
                t2s = tp.tile([128, L], BF16, tag="bigB")
                nc.vector.tensor_mul(t2s[:, :], O[:, :], sin_t[:, :])
                rE = tp.tile([128, L], BF16, tag="bigA")
                nc.vector.tensor_sub(rE[:, :], t1c[:, :], t2s[:, :])
                t1s = tp.tile([128, L], BF16, tag="bigC")
                nc.vector.tensor_mul(t1s[:, :], E[:, :], sin_t[:, :])
                t2c = tp.tile([128, L], BF16, tag="bigB")
                nc.vector.tensor_mul(t2c[:, :], O[:, :], cos_t[:, :])
                rO = t1s
                nc.vector.tensor_add(rO[:, :], t1s[:, :], t2c[:, :])
                sE = tp.tile([128, L], BF16, tag="sc16E")
                sO = tp.tile([128, L], BF16, tag="sc16O")
                for kind, (rt, st) in enumerate(((rE, sE), (rO, sO))):
                    gsl = gind_t[:, kind * 128:(kind + 1) * 128]
                    for lw in range(4):
                        mm = psp.tile([128, 512], F32, tag="mps")
                        nc.tensor.matmul(mm[:, :], gsl,
                                         invrs[lw][:, :],
                                         start=True, stop=True)
                        nc.vector.tensor_mul(
                            st[:, lw * 512:(lw + 1) * 512],
                            rt[:, lw * 512:(lw + 1) * 512], mm[:, :])
                for blk in range(2):
                    nc.sync.dma_start(out=qT[0][blk * 64:blk * 64 + 32, :],
                                      in_=sE[blk * 32:(blk + 1) * 32, :])
                    nc.sync.dma_start(out=qT[0][blk * 64 + 32:blk * 64 + 64, :],
                                      in_=sO[blk * 32:(blk + 1) * 32, :])
                    nc.sync.dma_start(out=kT[0][blk * 64:blk * 64 + 32, :],
                                      in_=sE[64 + blk * 32:64 + (blk + 1) * 32, :])
                    nc.sync.dma_start(out=kT[0][blk * 64 + 32:blk * 64 + 64, :],
                                      in_=sO[64 + blk * 32:64 + (blk + 1) * 32, :])
                # v transpose -> vseq [l,130]: [vA(64) 1 vB(64) 1]
                for lw in range(NKV):
                    pt = psp.tile([128, 128], BF16, tag="ptr")
                    nc.tensor.transpose(pt[:, :],
                                        V[:, lw * 128:(lw + 1) * 128],
                                        ident[:, :])
                    vv = vseq[0][lw].rearrange("a (h x) -> a h x", h=2)
                    nc.vector.tensor_copy(
                        vv[:, :, 0:64],
                        pt[:, :].rearrange("a (h x) -> a h x", h=2))
                    nc.vector.tensor_copy(vv[:, :, 64], onecol[:, :])

            chk.release()
            # pair0 half of the output projection, bf16 partials
            pav = tc.alloc_tile_pool(name="pav", bufs=1)
            pA = [[pav.tile([128, 512], BF16, tag=f"pA{qb}_{i}",
                            name=f"pA{qb}_{i}") for i in range(8)]
                  for qb in range(NQB)]
            dfr2 = tc.alloc_tile_pool(name="dfr2", bufs=1)
            rA = dfr2.tile([128, L], BF16, tag="rA")
            rB = dfr2.tile([128, L], BF16, tag="rB")
            sE1 = dfr2.tile([128, L], BF16, tag="sE1")
            sO1 = dfr2.tile([128, L], BF16, tag="sO1")
            lnv1 = [dfr2.tile([4, 512], F32, tag=f"lnv1_{lw}",
                              name=f"lnv1_{lw}") for lw in range(4)]
            iv1 = [dfr2.tile([4, 512], F32R, tag=f"iv1_{lw}",
                             name=f"iv1_{lw}") for lw in range(4)]
            E1, O1, V1 = chunks[3], chunks[4], chunks[5]

            # pair1 prep, dripped one item per attention window of pair0.
            # PSUM scratch comes from the attention-phase "po" slots, so the
            # closures must run inside the attention pool scope.
            # Filler ordering matters: engines execute in program order, so a
            # filler op whose producer (on another engine) hasn't run yet
            # head-blocks every later op on its engine — including the EXP
            # stream. Squares go first; ACT-free transpose items pad the gap
            # until the squares have certainly retired; only then the
            # Ln/Exp stats; then the gamma broadcasts; then rope.
            def _mk_deferred(pso_tile):
                u = []

                def qkv1(oc, lq):
                    ps = pso_tile([128, 512], F32)
                    for cc in range(8):
                        nc.tensor.matmul(
                            ps[:, :],
                            wq_sb[cc][:, oc * 128:(oc + 1) * 128],
                            xt_sb[cc][lq][:, :],
                            start=(cc == 0), stop=(cc == 7),
                        )
                    nc.vector.tensor_scalar_add(
                        chunks[oc][:, lq * 512:(lq + 1) * 512],
                        ps[:, :], bias_t[:, oc:oc + 1])
                for oc in (3, 4, 5):
                    for lq in range(4):
                        u.append((1700, lambda oc=oc, lq=lq: qkv1(oc, lq)))

                u.append((0, lambda: nc.vector.tensor_mul(rA[:, :], E1[:, :], E1[:, :])))
                u.append((0, lambda: nc.vector.tensor_mul(rB[:, :], O1[:, :], O1[:, :])))

                def stats_ln():
                    # one item: all four Ln calls back-to-back (1 table load)
                    for lw in range(4):
                        ps4 = pso_tile([4, 512], F32)
                        nc.tensor.matmul(ps4[:, :], ind_t[:, :],
                                         rA[:, lw * 512:(lw + 1) * 512],
                                         start=True, stop=False)
                        nc.tensor.matmul(ps4[:, :], ind_t[:, :],
                                         rB[:, lw * 512:(lw + 1) * 512],
                                         start=False, stop=True)
                        nc.scalar.activation(lnv1[lw][:, :], ps4[:, :], AF.Ln,
                                             scale=1.0 / 64.0,
                                             bias=eps_t[:, 0:1])

                def stats_exp():
                    for lw in range(4):
                        nc.scalar.activation(iv1[lw][:, :], lnv1[lw][:, :],
                                             AF.Exp, scale=-0.5,
                                             bias=lnb_t[:, 0:1])

                def msb(kind, lw):
                    gsl = gind_t[:, (2 + kind) * 128:(2 + kind + 1) * 128]
                    mm = pso_tile([128, 512], F32)
                    nc.tensor.matmul(mm[:, :], gsl, iv1[lw][:, :],
                                     start=True, stop=True)
                    nc.vector.tensor_copy(
                        M_sb[kind][:, lw * 512:(lw + 1) * 512], mm[:, :])

                def vtr(lw):
                    pt = pso_tile([128, 128], BF16)
                    nc.tensor.transpose(pt[:, :], V1[:, lw * 128:(lw + 1) * 128],
                                        ident[:, :])
                    vv = vseq[1][lw].rearrange("a (h x) -> a h x", h=2)
                    nc.vector.tensor_copy(
                        vv[:, :, 0:64],
                        pt[:, :].rearrange("a (h x) -> a h x", h=2))
                    nc.vector.tensor_copy(vv[:, :, 64], onecol[:, :])

                for lw in range(10):
                    u.append((100, lambda lw=lw: vtr(lw)))
                u.append((900, stats_ln))
                for lw in range(10, 13):
                    u.append((100, lambda lw=lw: vtr(lw)))
                u.append((0, stats_exp))
                for lw in range(13, NKV):
                    u.append((100, lambda lw=lw: vtr(lw)))
                for kind in range(2):
                    for lw in range(4):
                        u.append((250, lambda k=kind, lw=lw: msb(k, lw)))

                u.append((0, lambda: nc.vector.tensor_mul(rA[:, :], E1[:, :], cos_t[:, :])))
                u.append((0, lambda: nc.vector.tensor_mul(rB[:, :], O1[:, :], sin_t[:, :])))
                u.append((0, lambda: nc.vector.tensor_sub(rA[:, :], rA[:, :], rB[:, :])))
                u.append((0, lambda: nc.vector.tensor_mul(sE1[:, :], rA[:, :], M_sb[0][:, :])))
                u.append((0, lambda: nc.vector.tensor_mul(rA[:, :], E1[:, :], sin_t[:, :])))
                u.append((0, lambda: nc.vector.tensor_mul(rB[:, :], O1[:, :], cos_t[:, :])))
                u.append((0, lambda: nc.vector.tensor_add(rA[:, :], rA[:, :], rB[:, :])))
                u.append((0, lambda: nc.vector.tensor_mul(sO1[:, :], rA[:, :], M_sb[1][:, :])))
                for blk in range(2):
                    for args in (
                        (qT[1][blk * 64:blk * 64 + 32, :], sE1[blk * 32:(blk + 1) * 32, :]),
                        (qT[1][blk * 64 + 32:blk * 64 + 64, :], sO1[blk * 32:(blk + 1) * 32, :]),
                        (kT[1][blk * 64:blk * 64 + 32, :], sE1[64 + blk * 32:64 + (blk + 1) * 32, :]),
                        (kT[1][blk * 64 + 32:blk * 64 + 64, :], sO1[64 + blk * 32:64 + (blk + 1) * 32, :]),
                    ):
                        u.append((0, lambda a=args: nc.sync.dma_start(out=a[0], in_=a[1])))
                u.append((0, lambda: chkD.release()))
                u.append((0, lambda: xw.release()))
                return u

            # ---- phase 3: attention + fused output projection ----
            # Per (pair, q-block of 512): flash loop over 16 kv chunks.
            # Emission is software-pipelined (scores[j+1] ahead of PV[j]) so
            # the PE stream never head-blocks on the EXP; the out-projection
            # for q-block qb is dripped into the following windows as PE
            # filler (keeps the HAM clock warm).
            with (
                tc.tile_pool(name="exp", bufs=6) as xp,
                tc.tile_pool(name="den", bufs=3) as dn,
                tc.tile_pool(name="oev", bufs=3) as oe,
                tc.tile_pool(name="psa", bufs=2, space="PSUM") as psa,
                tc.tile_pool(name="psv", bufs=1, space="PSUM") as psv,
                tc.tile_pool(name="pso", bufs=2, space="PSUM") as pso,
            ):
                _poctr = [0]

                def _po_tile(shape, dt):
                    _poctr[0] += 1
                    return pso.tile(shape, dt, tag="po",
                                    name=f"pof{_poctr[0]}")

                filler = _mk_deferred(_po_tile)

                # out-projection split across pairs: pair0's half runs as
                # filler during its own attention (PSUM -> bf16 SBUF
                # partials), pair1's half adds them back. Both phases get a
                # real matmul per window, which keeps the PE HAM clock warm.
                def mk_proj0(qb):
                    def one(i, lw, n):
                        def run():
                            po = _po_tile([128, 512], F32)
                            nc.tensor.matmul(
                                po[:, :],
                                oq[0][qb][:, lw * 128:(lw + 1) * 128],
                                wo_t[0][:, n * 512:(n + 1) * 512],
                                start=True, stop=True)
                            nc.vector.tensor_copy(pA[qb][i][:, :], po[:, :])
                        return run
                    return [(450, one(i, lw, n))
                            for i, (lw, n) in enumerate(
                                (lw, n) for lw in range(4) for n in range(2))]

                def mk_proj1(qb):
                    def one(i, lw, n):
                        def run():
                            po = _po_tile([128, 512], F32)
                            nc.tensor.matmul(
                                po[:, :],
                                oq[1][qb][:, lw * 128:(lw + 1) * 128],
                                wo_t[1][:, n * 512:(n + 1) * 512],
                                start=True, stop=True)
                            ov = oe.tile([128, 512], F16, tag="ov")
                            nc.vector.tensor_add(ov[:, :], po[:, :],
                                                 pA[qb][i][:, :])
                            nc.sync.dma_start(
                                out=out_d[qb * LQB + lw * 128:
                                          qb * LQB + (lw + 1) * 128,
                                          n * 512:(n + 1) * 512],
                                in_=ov[:, :])
                        return run
                    return [(450, one(i, lw, n))
                            for i, (lw, n) in enumerate(
                                (lw, n) for lw in range(4) for n in range(2))]

                for p in range(NPAIR):
                    if p == 1:
                        while filler:
                            filler.pop(0)[1]()
                    for qb in range(NQB):
                        q0 = qb * LQB
                        oA = psv.tile([65, LQB], F32, tag="oA")
                        oB = psv.tile([65, LQB], F32, tag="oB")

                        def emit_scores(j):
                            sAB = psa.tile([128, 2 * LQB], F32, tag="sAB")
                            nc.tensor.matmul(
                                sAB[:, 0:LQB],
                                kT[p][0:64, j * 128:(j + 1) * 128],
                                qT[p][0:64, q0:q0 + LQB],
                                start=True, stop=True, tile_position=(0, 0))
                            nc.tensor.matmul(
                                sAB[:, LQB:2 * LQB],
                                kT[p][64:128, j * 128:(j + 1) * 128],
                                qT[p][64:128, q0:q0 + LQB],
                                start=True, stop=True, tile_position=(64, 0))
                            return sAB

                        def emit_pv(j, eAB):
                            nc.tensor.matmul(
                                oA[:, :], vseq[p][j][:, 0:65],
                                eAB[:, 0:LQB],
                                start=(j == 0), stop=(j == NKV - 1))
                            nc.tensor.matmul(
                                oB[:, :], vseq[p][j][:, 65:130],
                                eAB[:, LQB:2 * LQB],
                                start=(j == 0), stop=(j == NKV - 1))

                        prev_e = None
                        for j in range(NKV):
                            sAB = emit_scores(j)
                            if prev_e is not None:
                                emit_pv(j - 1, prev_e)
                            eAB = xp.tile([128, 2 * LQB], BF16, tag="eAB")
                            nc.scalar.activation(eAB[:, :], sAB[:, :], AF.Exp)
                            prev_e = eAB
                            got = 0
                            if filler:
                                w, fn = filler.pop(0)
                                fn()
                                got += w
                            if len(filler) > 16:
                                w, fn = filler.pop(0)
                                fn()
                                got += w
                        emit_pv(NKV - 1, prev_e)

                        # normalize: o / denom (denom = row 64); the
                        # ones-row matmul broadcast beats a DMA+GpSimd route
                        # (the DMA round-trip stalls each q-block boundary)
                        for hd, oo in enumerate((oA, oB)):
                            den = dn.tile([65, LQB], F32R, tag="den")
                            nc.vector.tensor_copy(den[64:65, :], oo[64:65, :])
                            dbc = pso.tile([64, LQB], F32, tag="po")
                            nc.tensor.matmul(
                                dbc[:, :], ones_t[64:65, :],
                                den[64:65, :], start=True, stop=True)
                            rcb = dn.tile([64, LQB], F32, tag="rcb")
                            nc.vector.reciprocal_approx_fast(rcb[:, :], dbc[:, :])
                            onrm = dn.tile([64, LQB], BF16, tag="onrm")
                            nc.vector.tensor_mul(onrm[:, :], oo[0:64, :], rcb[:, :])
                            nc.sync.dma_start(
                                out=oq[p][qb][hd * 64:(hd + 1) * 64, :],
                                in_=onrm[:, :])
                        filler.extend(
                            (mk_proj0 if p == 0 else mk_proj1)(qb))
                while filler:
                    filler.pop(0)[1]()


            dfr2.release()
            pav.release()
            dfr.release()
            lv.release()
    nc.compile()
    return nc


_PROG = None


def _get_program():
    global _PROG
    if _PROG is None:
        _PROG = _build_program()
    return _PROG


def _core_inputs(core, x, w_qkv, b_qkv, w_out, q_gamma, k_gamma,
                 cos_tab, sin_tab, ind, ones128):
    b = core // 4
    hb = (core % 4) * 4
    # row permutation of w_qkv for this core
    perm = []
    for p in range(NPAIR):
        hA, hB = hb + 2 * p, hb + 2 * p + 1
        for h in (hA, hB):                      # E chunk: q evens, k evens
            perm += [h * 64 + 2 * j for j in range(32)]
        for h in (hA, hB):
            perm += [1024 + h * 64 + 2 * j for j in range(32)]
        for h in (hA, hB):                      # O chunk
            perm += [h * 64 + 2 * j + 1 for j in range(32)]
        for h in (hA, hB):
            perm += [1024 + h * 64 + 2 * j + 1 for j in range(32)]
        for h in (hA, hB):                      # V chunk
            perm += [2048 + h * 64 + c for c in range(64)]
    perm = np.asarray(perm)
    w_local = w_qkv[perm]                       # [768, 1024]
    wq = np.ascontiguousarray(w_local.T)        # [1024, 768]
    bias6 = np.ascontiguousarray(b_qkv[perm].reshape(6, 128))

    # gamma-scaled indicator lhsT [4, 4*128]: (p, kind) -> [4, 128]
    gind = np.zeros((4, 4 * 128), np.float32)
    for p in range(NPAIR):
        for kind in range(2):                   # 0=E(evens), 1=O(odds)
            blk = (2 * p + kind) * 128
            for r in range(128):
                g = r // 32
                h = hb + 2 * p + (g % 2)
                ch = 2 * (r % 32) + kind
                gam = q_gamma[h, ch] if g < 2 else k_gamma[h, ch]
                gind[g, blk + r] = gam

    # w_out slice: [256, 1024]
    wo = np.empty((256, 1024), np.float32)
    for p in range(NPAIR):
        for i, h in enumerate((hb + 2 * p, hb + 2 * p + 1)):
            wo[p * 128 + i * 64:p * 128 + (i + 1) * 64, :] = \
                w_out[:, h * 64:(h + 1) * 64].T

    return {
        "xt": np.ascontiguousarray(x[b].T).astype(ml_dtypes.bfloat16),
        "wq": wq.astype(ml_dtypes.bfloat16),
        "bias6": bias6,
        "cost": cos_tab,
        "sint": sin_tab,
        "ind": ind.astype(ml_dtypes.bfloat16),
        "gind": gind,
        "ones128": ones128,
        "wo": wo.astype(ml_dtypes.bfloat16),
    }


def kernel(x, w_qkv, b_qkv, w_out, b_out, q_gamma, k_gamma, _trace=False):
    x = np.asarray(x, np.float32)
    w_qkv = np.asarray(w_qkv, np.float32)
    b_qkv = np.asarray(b_qkv, np.float32)
    w_out = np.asarray(w_out, np.float32)
    b_out = np.asarray(b_out, np.float32)
    q_gamma = np.asarray(q_gamma, np.float32)
    k_gamma = np.asarray(k_gamma, np.float32)

    inv_freq = (1.0 / ROPE_THETA ** (np.arange(32, dtype=np.float64) / 32.0))
    ang = np.arange(L, dtype=np.float64)[None, :] * \
        np.tile(inv_freq, 4)[:, None]          # [128, L], row r -> freq r%32
    cos_tab = np.cos(ang).astype(ml_dtypes.bfloat16)
    sin_tab = np.sin(ang).astype(ml_dtypes.bfloat16)
    ind = np.zeros((128, 4), np.float32)
    for r in range(128):
        ind[r, r // 32] = 1.0
    ones128 = np.ones((128, 64), np.float32)

    nc = _get_program()
    in_maps = [_core_inputs(c, x, w_qkv, b_qkv, w_out, q_gamma, k_gamma,
                            cos_tab, sin_tab, ind, ones128)
               for c in range(NCORES)]
    r = run_bass_kernel_spmd(nc, in_maps, list(range(NCORES)), trace=_trace)
    out = np.zeros((B, L, C), np.float32)
    for c in range(NCORES):
        out[c // 4] += r.results[c]["out"].astype(np.float32)
    out += b_out[None, None, :]
    if _trace:
        kernel._last_results = r
    return out

